# revision 4
# baseline (speedup 1.0000x reference)
"""Causal self-attention (GQA, RoPE, QK-RMSNorm) Trainium2 Bass kernel.

Sharding: tensor-parallel over heads x data-parallel over batch.
8 cores = 2 batch-groups (2 batches each) x 4 head-groups
(4 q heads + 1 kv head per core, GQA nrep=4).

Per core:
  - phase A: QKV projection (x^T via DMA-transpose as stationary operand),
    RoPE + RMSNorm in natural [t, d] layout, PE-transpose q,k to [d, t].
  - phase B: per (batch, j-chunk, head) "PJ": s^T = k^T.T @ q^T tiles ->
    exp on ACT (no max subtraction; rmsnormed q,k bound scores) with
    causally-trimmed widths -> boundary-block mask multiply -> E_sum
    (bf16 DVE adds) -> denominators via Pool partition_all_reduce ->
    reciprocal. AV is v-STATIONARY: psum[d, tq512] accumulates
    v[t0].T @ et[t0] over t0 (y^T lands pre-transposed, no ones column,
    no per-tqc psum juggling); per-colblock stop-flag splits keep the
    diagonal accumulation trimmed. yT_sb write folds the 1/D multiply.
  - phase C: partial out = y^T.T @ Wo_shard -> DRAM.
Host sums the 4 head-group partials per batch-group.

All matmuls bf16 inputs / fp32 PSUM accumulation; softmax, RoPE and
RMSNorm arithmetic in fp32 (E_sum accumulates in bf16; its ~0.3% rms
error on D is well inside the 2e-2 gate). rsqrt computed as
exp(-0.5*ln(x)) so every ACT call stays in one table set.
"""

import sys
from contextlib import ExitStack

import numpy as np

sys.path.insert(0, "/opt/trn_rl_repo")

import concourse.bass as bass  # noqa: E402
import concourse.bass_isa as bass_isa  # noqa: E402
import concourse.mybir as mybir  # noqa: E402
import concourse.tile as tile  # noqa: E402
from concourse import bacc  # noqa: E402

FP32 = mybir.dt.float32
BF16 = mybir.dt.bfloat16
P = 128
HD = 128
HD2 = HD // 2
TQW = 512  # tq chunk width for the QK stage


def build_nc(T=2048, C=2048, NHL=4, BL=2, repeat=1, et_bufs=2,
             xt_bufs=2, outsb_bufs=2, ps_s_bufs=3, ps_mm_bufs=2,
             drain_per_tile=1, no_qkt=0, no_av=0):
    """Build the per-core Bass program. Same program on all 8 cores."""
    TL = BL * T
    NCC = C // P  # contraction chunks
    NTB = T // P  # t-128 chunks per batch
    DQ = NHL * HD
    NJ = T // TQW  # tq-512 chunks per batch
    NK = T // P  # tk-128 chunks per batch
    NCO = C // 512
    sm_scale = float(1.0 / np.sqrt(HD))
    EPS = 1e-6

    nc = bacc.Bacc(None)
    x = nc.declare_dram_parameter("x", [C, TL], BF16, isOutput=False)
    wq = nc.declare_dram_parameter("wq", [C, DQ], BF16, isOutput=False)
    wkv = nc.declare_dram_parameter("wkv", [C, 2 * HD], BF16, isOutput=False)
    wo = nc.declare_dram_parameter("wo", [DQ, C], BF16, isOutput=False)
    cs = nc.declare_dram_parameter("cs", [T, HD], FP32, isOutput=False)
    sc = nc.declare_dram_parameter("sc", [T, HD], FP32, isOutput=False)
    masks = nc.declare_dram_parameter("masks", [P, P], BF16, isOutput=False)
    ident = nc.declare_dram_parameter("ident", [P, P], BF16, isOutput=False)
    out = nc.declare_dram_parameter("out", [TL, C], BF16, isOutput=True)

    Exp = mybir.ActivationFunctionType.Exp
    Ln = mybir.ActivationFunctionType.Ln
    xr = x.rearrange("(n p) t -> p n t", p=P)  # x is shipped pre-transposed [C, TL]

    with tile.TileContext(nc) as tc, ExitStack() as ctx:
        const = ctx.enter_context(tc.tile_pool(name="const", bufs=1))
        persist = ctx.enter_context(tc.tile_pool(name="persist", bufs=1))
        xt_pool = ctx.enter_context(tc.tile_pool(name="xt", bufs=xt_bufs))
        work = ctx.enter_context(tc.tile_pool(name="work", bufs=2))
        et_pool = ctx.enter_context(tc.tile_pool(name="et", bufs=et_bufs))
        outsb = ctx.enter_context(tc.tile_pool(name="outsb", bufs=outsb_bufs))
        ps_mm = ctx.enter_context(tc.tile_pool(name="psmm", bufs=ps_mm_bufs, space="PSUM"))
        ps_s = ctx.enter_context(tc.tile_pool(name="pss", bufs=ps_s_bufs, space="PSUM"))
        ps_y = ctx.enter_context(tc.tile_pool(name="psy", bufs=2, space="PSUM"))

        # ---- constants / persistent state ----
        wq_sb = const.tile([P, NCC, DQ], BF16)
        wqr = wq.rearrange("(n p) d -> p n d", p=P)
        nc.scalar.dma_start(wq_sb[:, 0 : NCC // 4, :], wqr[:, 0 : NCC // 4, :])
        nc.scalar.dma_start(wq_sb[:, NCC // 4 :, :], wqr[:, NCC // 4 :, :])
        wkv_sb = const.tile([P, NCC, 2 * HD], BF16)
        nc.scalar.dma_start(wkv_sb[:], wkv.rearrange("(n p) d -> p n d", p=P))
        cs_sb = const.tile([P, NTB, HD], FP32)
        nc.scalar.dma_start(cs_sb[:], cs.rearrange("(n p) d -> p n d", p=P))
        sc_sb = const.tile([P, NTB, HD], FP32)
        nc.scalar.dma_start(sc_sb[:], sc.rearrange("(n p) d -> p n d", p=P))
        ident_sb = const.tile([P, P], BF16)
        nc.scalar.dma_start(ident_sb[:], ident[:])
        mask_sb = const.tile([P, P], BF16)
        nc.scalar.dma_start(mask_sb[:], masks[:])
        wo_sb = const.tile([P, NHL, C], BF16)
        nc.scalar.dma_start(wo_sb[:], wo.rearrange("(h p) c -> p h c", p=P))
        eps_sb = const.tile([P, 1], FP32)
        nc.vector.memset(eps_sb[:], EPS)

        qT_sb = persist.tile([P, BL * NHL, T], BF16)  # [d, plane, t]
        kT_sb = persist.tile([P, BL, T], BF16)
        v_sb = persist.tile([P, BL, NK, HD], BF16)  # [tk_in, b, tk_chunk, d]
        yT_sb = persist.tile([P, BL * NHL, T], BF16)

        if no_qkt:
            nc.vector.memset(qT_sb[:], 0.0)
            nc.vector.memset(kT_sb[:], 0.0)
        if no_av:
            nc.vector.memset(yT_sb[:], 0.0)

        rep_ctx = tc.For_i(0, repeat, 1) if repeat > 1 else None
        if rep_ctx is not None:
            ctx.enter_context(rep_ctx)

        # ---- phase A iteration: projection + rope + rmsnorm + transpose ----
        def a_iter(b, t16):
            trow = b * T + t16 * P
            xt = xt_pool.tile([P, NCC, P], BF16, tag="xt")
            nc.sync.dma_start(xt[:], xr[:, :, trow : trow + P])

            psq = ps_mm.tile([P, 512], FP32, tag="mm512")
            pskv = ps_mm.tile([P, 512], FP32, tag="mm512")
            for c in range(NCC):
                nc.tensor.matmul(
                    psq[:, :DQ], xt[:, c, :], wq_sb[:, c, :],
                    start=(c == 0), stop=(c == NCC - 1),
                )
            for c in range(NCC):
                nc.tensor.matmul(
                    pskv[:, 0 : 2 * HD], xt[:, c, :], wkv_sb[:, c, :],
                    start=(c == 0), stop=(c == NCC - 1),
                )

            # rope: p1 = [x1*cos | x2*sin], p2 = [x1*sin | x2*cos]
            # y1 = p1_lo + p1_hi ; y2 = p2_hi - p2_lo
            psq3 = psq[:, 0:DQ].rearrange("p (h d) -> p h d", d=HD)
            csb = cs_sb[:, t16, None, :].to_broadcast((P, NHL, HD))
            scb = sc_sb[:, t16, None, :].to_broadcast((P, NHL, HD))
            p1 = work.tile([P, NHL, HD], FP32, tag="p1")
            p2 = work.tile([P, NHL, HD], FP32, tag="p2")
            nc.vector.tensor_mul(p1[:], psq3, csb)
            nc.vector.tensor_mul(p2[:], psq3, scb)
            q_ro = work.tile([P, NHL, HD], FP32, tag="qro")
            nc.vector.tensor_add(
                q_ro[:, :, 0:HD2], p1[:, :, 0:HD2], p1[:, :, HD2:HD]
            )
            nc.vector.tensor_sub(
                q_ro[:, :, HD2:HD], p2[:, :, HD2:HD], p2[:, :, 0:HD2]
            )
            p1k = work.tile([P, HD], FP32, tag="p1k")
            p2k = work.tile([P, HD], FP32, tag="p2k")
            nc.vector.tensor_mul(p1k[:], pskv[:, 0:HD], cs_sb[:, t16, :])
            nc.vector.tensor_mul(p2k[:], pskv[:, 0:HD], sc_sb[:, t16, :])
            k_ro = work.tile([P, HD], FP32, tag="kro")
            nc.vector.tensor_add(k_ro[:, 0:HD2], p1k[:, 0:HD2], p1k[:, HD2:HD])
            nc.vector.tensor_sub(k_ro[:, HD2:HD], p2k[:, HD2:HD], p2k[:, 0:HD2])

            # rmsnorm scales: rs = exp(-0.5 * ln(ssq/HD + eps))
            sq = work.tile([P, NHL, HD], FP32, tag="p1")  # reuse p1 ring (SBUF)
            nc.vector.tensor_mul(sq[:], q_ro[:], q_ro[:])
            sqk = work.tile([P, HD], FP32, tag="sqk")
            nc.vector.tensor_mul(sqk[:], k_ro[:], k_ro[:])
            ssq = work.tile([P, NHL + 1], FP32, tag="ssq")
            nc.vector.tensor_reduce(
                ssq[:, 0:NHL], sq[:], axis=mybir.AxisListType.X,
                op=mybir.AluOpType.add,
            )
            nc.vector.tensor_reduce(
                ssq[:, NHL : NHL + 1], sqk[:], axis=mybir.AxisListType.X,
                op=mybir.AluOpType.add,
            )
            lnv = work.tile([P, NHL + 1], FP32, tag="lnv")
            nc.scalar.activation(
                lnv[:], ssq[:], Ln, bias=eps_sb[:, 0:1], scale=1.0 / HD
            )
            rs = work.tile([P, NHL + 1], FP32, tag="rs")
            nc.scalar.activation(rs[:], lnv[:], Exp, scale=-0.5)

            q_n = work.tile([P, NHL, HD], BF16, tag="qn")
            nc.vector.tensor_mul(
                q_n[:], q_ro[:], rs[:, 0:NHL, None].to_broadcast((P, NHL, HD))
            )
            k_n = work.tile([P, HD], BF16, tag="kn")
            nc.vector.tensor_scalar_mul(k_n[:], k_ro[:], rs[:, NHL : NHL + 1])

            nc.vector.tensor_copy(v_sb[:, b, t16, 0:HD], pskv[:, HD : 2 * HD])

            if no_qkt:
                return
            pt = ps_y.tile([P, NHL + 1, 132], BF16, tag="ptq", bufs=1)
            for h in range(NHL):
                nc.tensor.transpose(pt[:, h, 0:P], q_n[:, h, :], ident_sb[:])
            nc.tensor.transpose(pt[:, NHL, 0:P], k_n[:], ident_sb[:])
            nc.any.tensor_copy(
                qT_sb[:, b * NHL : b * NHL + NHL, t16 * P : (t16 + 1) * P],
                pt[:, 0:NHL, 0:P],
            )
            nc.any.tensor_copy(
                kT_sb[:, b, t16 * P : (t16 + 1) * P], pt[:, NHL, 0:P]
            )

        # ---- phase C: one output-column chunk of Wo for one (b, t16) ----
        def c_co(b, t16, co):
            pso = ps_mm.tile([P, 512], FP32, tag="mm512", name=f"pso{co}")
            for h in range(NHL):
                nc.tensor.matmul(
                    pso[:],
                    yT_sb[:, b * NHL + h, t16 * P : (t16 + 1) * P],
                    wo_sb[:, h, co * 512 : (co + 1) * 512],
                    start=(h == 0), stop=(h == NHL - 1),
                )
            o_sb = outsb.tile([P, 512], BF16, tag="osb")
            nc.any.tensor_copy(o_sb[:], pso[:])
            trow = b * T + t16 * P
            nc.sync.dma_start(
                out[trow : trow + P, co * 512 : (co + 1) * 512], o_sb[:]
            )

        # ---- phase B: scores+exp+denominator for one PJ=(b,j,h) ----
        def b_scores(b, j, h, av_gen):
            """Emit score matmuls + exp + mask + E_sum for PJ (b,j,h),
            interleaving the pending AV matmul stream (av_gen) of the
            previous PJ so PE has work while ACT chews through the exps.
            Ends with the denominator chain (Pool allreduce + reciprocal).
            Returns (et_slot, rcp_tile, av_gen)."""
            plane = b * NHL + h
            ntiles = 4 * j + 4
            et = et_pool.tile([P, 4 * NJ, TQW], BF16, tag="et")
            es = work.tile([P, TQW], BF16, tag="esum")
            for t0 in range(ntiles):
                m = t0 - 4 * j
                c0 = max(m, 0) * P
                ps = ps_s.tile([P, TQW], FP32, tag="s")
                nc.tensor.matmul(
                    ps[:, c0:TQW],
                    kT_sb[:, b, t0 * P : (t0 + 1) * P],
                    qT_sb[:, plane, j * TQW + c0 : (j + 1) * TQW],
                    start=True, stop=True,
                )
                nc.scalar.activation(
                    et[:, t0, c0:TQW], ps[:, c0:TQW], Exp, scale=sm_scale
                )
                if m >= 0:
                    nc.vector.tensor_mul(
                        et[:, t0, c0 : c0 + P], et[:, t0, c0 : c0 + P],
                        mask_sb[:],
                    )
                if t0 == 0:
                    nc.vector.tensor_copy(es[:], et[:, 0, :])
                else:
                    nc.vector.tensor_add(
                        es[:, c0:TQW], es[:, c0:TQW], et[:, t0, c0:TQW]
                    )
                if av_gen is not None:
                    for _ in range(drain_per_tile):
                        if next(av_gen, None) is None:
                            av_gen = None
                            break
            dbc = work.tile([P, TQW], FP32, tag="dbc")
            nc.gpsimd.partition_all_reduce(
                dbc[:], es[:], channels=P, reduce_op=bass_isa.ReduceOp.add
            )
            rcp = work.tile([P, TQW], FP32, tag="rcp")
            nc.vector.reciprocal(rcp[:], dbc[:])
            return et, rcp, av_gen

        def b_av_gen(b, j, h, et, rcp):
            """v-stationary AV: psum[d, tq512] accumulates v[t0].T @ et[t0]
            with causal trimming; diagonal tiles split per colblock so the
            stop flags close each column range exactly once. Ends with the
            1/D-folded copy into yT_sb. Yields after each matmul."""
            plane = b * NHL + h
            ntiles = 4 * j + 4
            psy = ps_y.tile([P, TQW], FP32, tag="yt")
            if no_av:
                return
            for t0 in range(ntiles):
                m = t0 - 4 * j
                c0 = max(m, 0) * P
                first = t0 == 0
                vst = v_sb[:, b, t0, :]
                if 0 <= m < 3:
                    nc.tensor.matmul(
                        psy[:, c0 : c0 + P], vst, et[:, t0, c0 : c0 + P],
                        start=first, stop=True,
                    )
                    yield True
                    nc.tensor.matmul(
                        psy[:, c0 + P : TQW], vst, et[:, t0, c0 + P : TQW],
                        start=first, stop=False,
                    )
                    yield True
                elif m == 3:
                    nc.tensor.matmul(
                        psy[:, c0:TQW], vst, et[:, t0, c0:TQW],
                        start=first, stop=True,
                    )
                    yield True
                else:
                    nc.tensor.matmul(
                        psy[:], vst, et[:, t0, :], start=first, stop=False,
                    )
                    yield True
            nc.vector.tensor_mul(
                yT_sb[:, plane, j * TQW : (j + 1) * TQW], psy[:], rcp[:]
            )
            yield True

        # ---- pipelined schedule ----
        # prologue: first A window of batch 0
        for t16 in range(4):
            a_iter(0, t16)

        c_queue = []  # ready (b, t16, co) phase-C chunks
        pending = None  # (av_gen, b, j) of the previous PJ
        plane_order = [(b, j, h) for b in range(BL) for j in range(NJ)
                       for h in range(NHL)]

        def drain_pending(push_c):
            nonlocal pending
            if pending is None:
                return
            gen, pb, pj = pending
            if gen is not None:
                for _ in gen:
                    pass
            pending = None
            if push_c:
                for t16 in range(4 * pj, 4 * pj + 4):
                    for co in range(NCO):
                        c_queue.append((pb, t16, co))

        for b, j, h in plane_order:
            av = pending[0] if pending is not None else None
            et, rcp, av = b_scores(b, j, h, av)
            if pending is not None:
                pending = (av, pending[1], pending[2])
                drain_pending(push_c=(h == 0))  # prev PJ was (b', j', h=3)
            pending = (b_av_gen(b, j, h, et, rcp), b, j)

            # filler slot: phase A iteration and/or phase C chunks
            if b == 0 and j < 3:
                a_iter(0, 4 * (j + 1) + h)
            elif b == 0 and j == 3:
                a_iter(1, h)
            elif b == 1 and j < 3:
                a_iter(1, 4 * (j + 1) + h)
            n_c = 3 if b == 0 else (4 if j < 3 else 6)
            for _ in range(min(n_c, len(c_queue))):
                c_co(*c_queue.pop(0))

        drain_pending(push_c=True)
        while c_queue:
            c_co(*c_queue.pop(0))

    nc.finalize()
    _collapse_act_table_loads(nc)
    return nc


def _collapse_act_table_loads(nc):
    """Every ACT function used here (Exp, Ln, Copy, Identity, Square) lives in
    the natural_log_exp_and_others set, but the insertion pass alternates
    exp_and_others / natural_log (one ~1.3us reload per rsqrt) — rewrite the
    first load to the combined set and drop the redundant reloads."""
    from concourse.hw_specs import get_activation_tables

    tables = list(get_activation_tables(nc.m.arch))
    target = tables.index("natural_log_exp_and_others")
    first = True
    for fn in nc.m.functions:
        for bb in fn.blocks:
            kept = []
            changed = False
            for inst in bb.instructions:
                if type(inst).__name__ == "InstLoadActFuncSet":
                    assert inst.sync_info is None
                    if first:
                        inst.act_func_set_id = target
                        first = False
                        kept.append(inst)
                    else:
                        changed = True
                    continue
                kept.append(inst)
            if changed:
                del bb.instructions[:]
                for i in kept:
                    bb.instructions.append(i)


def make_host_consts(T, dtype_bf):
    """Boundary causal mask (keep cc >= i) + PE-transpose identity."""
    i = np.arange(P)[:, None]
    cc = np.arange(P)[None, :]
    mask = (i <= cc).astype(np.float32).astype(dtype_bf)
    ident = np.eye(P, dtype=np.float32).astype(dtype_bf)
    return mask, ident


def prepare_in_maps(x, cos, sin, Wq, Wk, Wv, Wo, T=2048, C=2048, NHL=4, BL=2):
    import ml_dtypes

    bf = ml_dtypes.bfloat16
    B = x.shape[0]
    n_bgrp = B // BL
    n_hgrp = (Wq.shape[1] // HD) // NHL
    DQ = NHL * HD

    x_bf = np.ascontiguousarray(x.astype(bf))
    cosf = np.ascontiguousarray(cos.reshape(T, HD2).astype(np.float32))
    sinf = np.ascontiguousarray(sin.reshape(T, HD2).astype(np.float32))
    cs = np.ascontiguousarray(np.concatenate([cosf, sinf], axis=1))
    sc = np.ascontiguousarray(np.concatenate([sinf, cosf], axis=1))
    mask, ident = make_host_consts(T, bf)

    in_maps = []
    for g in range(n_bgrp):
        x_sh = np.ascontiguousarray(x_bf[BL * g : BL * (g + 1)].reshape(BL * T, C).T)
        for hg in range(n_hgrp):
            in_maps.append(
                {
                    "x": x_sh,
                    "wq": np.ascontiguousarray(
                        Wq[:, DQ * hg : DQ * (hg + 1)].astype(bf)
                    ),
                    "wkv": np.ascontiguousarray(
                        np.concatenate(
                            [
                                Wk[:, HD * hg : HD * (hg + 1)],
                                Wv[:, HD * hg : HD * (hg + 1)],
                            ],
                            axis=1,
                        ).astype(bf)
                    ),
                    "wo": np.ascontiguousarray(
                        Wo[DQ * hg : DQ * (hg + 1), :].astype(bf)
                    ),
                    "cs": cs,
                    "sc": sc,
                    "masks": mask,
                    "ident": ident,
                }
            )
    return in_maps


def run_on_device(x, cos, sin, Wq, Wk, Wv, Wo, trace=False):
    from concourse.bass_utils import run_bass_kernel_spmd

    T, C, NHL, BL = 2048, 2048, 4, 2
    in_maps = prepare_in_maps(x, cos, sin, Wq, Wk, Wv, Wo, T, C, NHL, BL)
    nc = build_nc(T, C, NHL, BL)
    res = run_bass_kernel_spmd(nc, in_maps, list(range(8)), trace=trace)

    B = x.shape[0]
    out = np.zeros((B, T, C), np.float32)
    n_hgrp = len(in_maps) // (B // BL)
    for g in range(B // BL):
        acc = np.zeros((BL * T, C), np.float32)
        for hg in range(n_hgrp):
            acc += res.results[g * n_hgrp + hg]["out"].astype(np.float32)
        out[BL * g : BL * (g + 1)] = acc.reshape(BL, T, C)
    return out, res


def kernel(x, cos, sin, Wq, Wk, Wv, Wo):
    out, _ = run_on_device(
        np.asarray(x), np.asarray(cos), np.asarray(sin),
        np.asarray(Wq), np.asarray(Wk), np.asarray(Wv), np.asarray(Wo),
    )
    return out


# revision 7
# speedup vs baseline: 1.1164x; 1.1164x over previous
"""Causal self-attention (GQA, RoPE, QK-RMSNorm) Trainium2 Bass kernel.

Sharding: tensor-parallel over heads x data-parallel over batch.
8 cores = 2 batch-groups (2 batches each) x 4 head-groups
(4 q heads + 1 kv head per core, GQA nrep=4).

Per core:
  - phase A: QKV projection (x^T via DMA-transpose as stationary operand),
    RoPE + RMSNorm in natural [t, d] layout, PE-transpose q,k to [d, t].
  - phase B: per (batch, j-chunk, head) "PJ": s^T = k^T.T @ q^T tiles ->
    exp on ACT (no max subtraction; rmsnormed q,k bound scores) with
    causally-trimmed widths -> boundary-block mask multiply -> E_sum
    (bf16 DVE adds) -> denominators via Pool partition_all_reduce ->
    reciprocal. AV is v-STATIONARY: psum[d, tq512] accumulates
    v[t0].T @ et[t0] over t0 (y^T lands pre-transposed, no ones column,
    no per-tqc psum juggling); per-colblock stop-flag splits keep the
    diagonal accumulation trimmed. yT_sb write folds the 1/D multiply.
  - phase C: partial out = y^T.T @ Wo_shard -> DRAM.
Host sums the 4 head-group partials per batch-group.

All matmuls bf16 inputs / fp32 PSUM accumulation; softmax, RoPE and
RMSNorm arithmetic in fp32 (E_sum accumulates in bf16; its ~0.3% rms
error on D is well inside the 2e-2 gate). rsqrt computed as
exp(-0.5*ln(x)) so every ACT call stays in one table set.
"""

import sys
from contextlib import ExitStack

import numpy as np

sys.path.insert(0, "/opt/trn_rl_repo")

import concourse.bass as bass  # noqa: E402
import concourse.bass_isa as bass_isa  # noqa: E402
import concourse.mybir as mybir  # noqa: E402
import concourse.tile as tile  # noqa: E402
from concourse import bacc  # noqa: E402

FP32 = mybir.dt.float32
BF16 = mybir.dt.bfloat16
P = 128
HD = 128
HD2 = HD // 2
TQW = 512  # tq chunk width for the QK stage


def build_nc(T=2048, C=2048, NHL=4, BL=2, repeat=1, et_bufs=2,
             xt_bufs=2, outsb_bufs=2, ps_s_bufs=3, ps_mm_bufs=2,
             drain_per_tile=1, no_qkt=0, no_av=0):
    """Build the per-core Bass program. Same program on all 8 cores."""
    TL = BL * T
    NCC = C // P  # contraction chunks
    NTB = T // P  # t-128 chunks per batch
    DQ = NHL * HD
    NJ = T // TQW  # tq-512 chunks per batch
    NK = T // P  # tk-128 chunks per batch
    NCO = C // 512
    sm_scale = float(1.0 / np.sqrt(HD))
    EPS = 1e-6

    nc = bacc.Bacc(None)
    x = nc.declare_dram_parameter("x", [C, TL], BF16, isOutput=False)
    wq = nc.declare_dram_parameter("wq", [C, DQ], BF16, isOutput=False)
    wkv = nc.declare_dram_parameter("wkv", [C, 2 * HD], BF16, isOutput=False)
    wo = nc.declare_dram_parameter("wo", [DQ, C], BF16, isOutput=False)
    cs = nc.declare_dram_parameter("cs", [T, HD], FP32, isOutput=False)
    sc = nc.declare_dram_parameter("sc", [T, HD], FP32, isOutput=False)
    masks = nc.declare_dram_parameter("masks", [P, P], BF16, isOutput=False)
    ident = nc.declare_dram_parameter("ident", [P, P], BF16, isOutput=False)
    out = nc.declare_dram_parameter("out", [TL, C], BF16, isOutput=True)

    Exp = mybir.ActivationFunctionType.Exp
    Ln = mybir.ActivationFunctionType.Ln
    xr = x.rearrange("(n p) t -> p n t", p=P)  # x is shipped pre-transposed [C, TL]

    with tile.TileContext(nc) as tc, ExitStack() as ctx:
        const = ctx.enter_context(tc.tile_pool(name="const", bufs=1))
        persist = ctx.enter_context(tc.tile_pool(name="persist", bufs=1))
        xt_pool = ctx.enter_context(tc.tile_pool(name="xt", bufs=xt_bufs))
        work = ctx.enter_context(tc.tile_pool(name="work", bufs=2))
        et_pool = ctx.enter_context(tc.tile_pool(name="et", bufs=et_bufs))
        outsb = ctx.enter_context(tc.tile_pool(name="outsb", bufs=outsb_bufs))
        ps_mm = ctx.enter_context(tc.tile_pool(name="psmm", bufs=ps_mm_bufs, space="PSUM"))
        ps_s = ctx.enter_context(tc.tile_pool(name="pss", bufs=ps_s_bufs, space="PSUM"))
        ps_y = ctx.enter_context(tc.tile_pool(name="psy", bufs=2, space="PSUM"))

        # ---- constants / persistent state ----
        wq_sb = const.tile([P, NCC, DQ], BF16)
        wqr = wq.rearrange("(n p) d -> p n d", p=P)
        nc.scalar.dma_start(wq_sb[:, 0 : NCC // 4, :], wqr[:, 0 : NCC // 4, :])
        nc.scalar.dma_start(wq_sb[:, NCC // 4 :, :], wqr[:, NCC // 4 :, :])
        wkv_sb = const.tile([P, NCC, 2 * HD], BF16)
        nc.scalar.dma_start(wkv_sb[:], wkv.rearrange("(n p) d -> p n d", p=P))
        cs_sb = const.tile([P, NTB, HD], FP32)
        nc.scalar.dma_start(cs_sb[:], cs.rearrange("(n p) d -> p n d", p=P))
        sc_sb = const.tile([P, NTB, HD], FP32)
        nc.scalar.dma_start(sc_sb[:], sc.rearrange("(n p) d -> p n d", p=P))
        ident_sb = const.tile([P, P], BF16)
        nc.scalar.dma_start(ident_sb[:], ident[:])
        mask_sb = const.tile([P, P], BF16)
        nc.scalar.dma_start(mask_sb[:], masks[:])
        ones_sb = const.tile([P, P], BF16)  # ones col [:,0:1] + row [0:1,:]
        nc.vector.memset(ones_sb[:], 1.0)
        wo_sb = const.tile([P, NHL, C], BF16)
        nc.scalar.dma_start(wo_sb[:], wo.rearrange("(h p) c -> p h c", p=P))
        eps_sb = const.tile([P, 1], FP32)
        nc.vector.memset(eps_sb[:], EPS)

        qT_sb = persist.tile([P, BL * NHL, T], BF16)  # [d, plane, t]
        kT_sb = persist.tile([P, BL, T], BF16)
        v_sb = persist.tile([P, BL, NK, HD], BF16)  # [tk_in, b, tk_chunk, d]
        yT_sb = persist.tile([P, BL * NHL, T], BF16)

        if no_qkt:
            nc.vector.memset(qT_sb[:], 0.0)
            nc.vector.memset(kT_sb[:], 0.0)
        if no_av:
            nc.vector.memset(yT_sb[:], 0.0)

        rep_ctx = tc.For_i(0, repeat, 1) if repeat > 1 else None
        if rep_ctx is not None:
            ctx.enter_context(rep_ctx)

        # ---- phase A iteration: projection + rope + rmsnorm + transpose ----
        def a_iter(b, t16):
            trow = b * T + t16 * P
            xt = xt_pool.tile([P, NCC, P], BF16, tag="xt")
            nc.sync.dma_start(xt[:], xr[:, :, trow : trow + P])

            psq = ps_mm.tile([P, 512], FP32, tag="mm512")
            pskv = ps_mm.tile([P, 512], FP32, tag="mm512")
            for c in range(NCC):
                nc.tensor.matmul(
                    psq[:, :DQ], xt[:, c, :], wq_sb[:, c, :],
                    start=(c == 0), stop=(c == NCC - 1),
                )
            for c in range(NCC):
                nc.tensor.matmul(
                    pskv[:, 0 : 2 * HD], xt[:, c, :], wkv_sb[:, c, :],
                    start=(c == 0), stop=(c == NCC - 1),
                )

            # rope: p1 = [x1*cos | x2*sin], p2 = [x1*sin | x2*cos]
            # y1 = p1_lo + p1_hi ; y2 = p2_hi - p2_lo
            psq3 = psq[:, 0:DQ].rearrange("p (h d) -> p h d", d=HD)
            csb = cs_sb[:, t16, None, :].to_broadcast((P, NHL, HD))
            scb = sc_sb[:, t16, None, :].to_broadcast((P, NHL, HD))
            p1 = work.tile([P, NHL, HD], FP32, tag="p1")
            p2 = work.tile([P, NHL, HD], FP32, tag="p2")
            nc.vector.tensor_mul(p1[:], psq3, csb)
            nc.vector.tensor_mul(p2[:], psq3, scb)
            q_ro = work.tile([P, NHL, HD], FP32, tag="qro")
            nc.vector.tensor_add(
                q_ro[:, :, 0:HD2], p1[:, :, 0:HD2], p1[:, :, HD2:HD]
            )
            nc.vector.tensor_sub(
                q_ro[:, :, HD2:HD], p2[:, :, HD2:HD], p2[:, :, 0:HD2]
            )
            p1k = work.tile([P, HD], FP32, tag="p1k")
            p2k = work.tile([P, HD], FP32, tag="p2k")
            nc.vector.tensor_mul(p1k[:], pskv[:, 0:HD], cs_sb[:, t16, :])
            nc.vector.tensor_mul(p2k[:], pskv[:, 0:HD], sc_sb[:, t16, :])
            k_ro = work.tile([P, HD], FP32, tag="kro")
            nc.vector.tensor_add(k_ro[:, 0:HD2], p1k[:, 0:HD2], p1k[:, HD2:HD])
            nc.vector.tensor_sub(k_ro[:, HD2:HD], p2k[:, HD2:HD], p2k[:, 0:HD2])

            # rmsnorm scales: rs = exp(-0.5 * ln(ssq/HD + eps))
            sq = work.tile([P, NHL, HD], FP32, tag="p1")  # reuse p1 ring (SBUF)
            nc.vector.tensor_mul(sq[:], q_ro[:], q_ro[:])
            sqk = work.tile([P, HD], FP32, tag="sqk")
            nc.vector.tensor_mul(sqk[:], k_ro[:], k_ro[:])
            ssq = work.tile([P, NHL + 1], FP32, tag="ssq")
            nc.vector.tensor_reduce(
                ssq[:, 0:NHL], sq[:], axis=mybir.AxisListType.X,
                op=mybir.AluOpType.add,
            )
            nc.vector.tensor_reduce(
                ssq[:, NHL : NHL + 1], sqk[:], axis=mybir.AxisListType.X,
                op=mybir.AluOpType.add,
            )
            lnv = work.tile([P, NHL + 1], FP32, tag="lnv")
            nc.scalar.activation(
                lnv[:], ssq[:], Ln, bias=eps_sb[:, 0:1], scale=1.0 / HD
            )
            rs = work.tile([P, NHL + 1], FP32, tag="rs")
            nc.scalar.activation(rs[:], lnv[:], Exp, scale=-0.5)

            q_n = work.tile([P, NHL, HD], BF16, tag="qn")
            nc.vector.tensor_mul(
                q_n[:], q_ro[:], rs[:, 0:NHL, None].to_broadcast((P, NHL, HD))
            )
            k_n = work.tile([P, HD], BF16, tag="kn")
            nc.vector.tensor_scalar_mul(k_n[:], k_ro[:], rs[:, NHL : NHL + 1])

            nc.vector.tensor_copy(v_sb[:, b, t16, 0:HD], pskv[:, HD : 2 * HD])

            if no_qkt:
                return
            pt = ps_y.tile([P, NHL + 1, 132], BF16, tag="ptq", bufs=1)
            for h in range(NHL):
                nc.tensor.transpose(pt[:, h, 0:P], q_n[:, h, :], ident_sb[:])
            nc.tensor.transpose(pt[:, NHL, 0:P], k_n[:], ident_sb[:])
            nc.any.tensor_copy(
                qT_sb[:, b * NHL : b * NHL + NHL, t16 * P : (t16 + 1) * P],
                pt[:, 0:NHL, 0:P],
            )
            nc.any.tensor_copy(
                kT_sb[:, b, t16 * P : (t16 + 1) * P], pt[:, NHL, 0:P]
            )

        # ---- phase C: one output-column chunk of Wo for one (b, t16) ----
        def c_co(b, t16, co):
            pso = ps_mm.tile([P, 512], FP32, tag="mm512", name=f"pso{co}")
            for h in range(NHL):
                nc.tensor.matmul(
                    pso[:],
                    yT_sb[:, b * NHL + h, t16 * P : (t16 + 1) * P],
                    wo_sb[:, h, co * 512 : (co + 1) * 512],
                    start=(h == 0), stop=(h == NHL - 1),
                )
            o_sb = outsb.tile([P, 512], BF16, tag="osb")
            nc.any.tensor_copy(o_sb[:], pso[:])
            trow = b * T + t16 * P
            nc.sync.dma_start(
                out[trow : trow + P, co * 512 : (co + 1) * 512], o_sb[:]
            )

        # ---- phase B: scores+exp+denominator for one PJ=(b,j,h) ----
        def b_scores(b, j, h, av_gen):
            """Emit score matmuls + exp + mask + E_sum for PJ (b,j,h),
            interleaving the pending AV matmul stream (av_gen) of the
            previous PJ so PE has work while ACT chews through the exps.
            Ends with the denominator chain (Pool allreduce + reciprocal).
            Returns (et_slot, rcp_tile, av_gen)."""
            plane = b * NHL + h
            ntiles = 4 * j + 4
            et = et_pool.tile([P, 4 * NJ, TQW], BF16, tag="et")
            es = work.tile([P, TQW], BF16, tag="esum")
            for t0 in range(ntiles):
                m = t0 - 4 * j
                c0 = max(m, 0) * P
                ps = ps_s.tile([P, TQW], FP32, tag="s")
                nc.tensor.matmul(
                    ps[:, c0:TQW],
                    kT_sb[:, b, t0 * P : (t0 + 1) * P],
                    qT_sb[:, plane, j * TQW + c0 : (j + 1) * TQW],
                    start=True, stop=True,
                )
                nc.scalar.activation(
                    et[:, t0, c0:TQW], ps[:, c0:TQW], Exp, scale=sm_scale
                )
                if m >= 0:
                    nc.vector.tensor_mul(
                        et[:, t0, c0 : c0 + P], et[:, t0, c0 : c0 + P],
                        mask_sb[:],
                    )
                if t0 == 0:
                    nc.vector.tensor_copy(es[:], et[:, 0, :])
                else:
                    nc.vector.tensor_add(
                        es[:, c0:TQW], es[:, c0:TQW], et[:, t0, c0:TQW]
                    )
                if av_gen is not None:
                    for _ in range(drain_per_tile):
                        if next(av_gen, None) is None:
                            av_gen = None
                            break
            # denominator: PE ones-matmul collapse -> bf16 row -> PE
            # ones-broadcast -> full-width reciprocal (all off Pool; the
            # gpsimd allreduce is ~5us/call on HW).
            d_ps = ps_s.tile([P, TQW], FP32, tag="s", name="dps")
            nc.tensor.matmul(
                d_ps[0:1, :], ones_sb[:, 0:1], es[:], start=True, stop=True
            )
            d_row = work.tile([1, TQW], BF16, tag="drow")
            nc.scalar.activation(
                d_row[:], d_ps[0:1, :], mybir.ActivationFunctionType.Copy
            )
            bc_ps = ps_s.tile([P, TQW], FP32, tag="s", name="bcps")
            nc.tensor.matmul(
                bc_ps[:], ones_sb[0:1, 0:P], d_row[:], start=True, stop=True
            )
            rcp = work.tile([P, TQW], FP32, tag="rcp")
            nc.vector.reciprocal(rcp[:], bc_ps[:])
            return et, rcp, av_gen

        def b_av_gen(b, j, h, et, rcp):
            """v-stationary AV: psum[d, tq512] accumulates v[t0].T @ et[t0]
            with causal trimming; diagonal tiles split per colblock so the
            stop flags close each column range exactly once. Ends with the
            1/D-folded copy into yT_sb. Yields after each matmul."""
            plane = b * NHL + h
            ntiles = 4 * j + 4
            psy = ps_y.tile([P, TQW], FP32, tag="yt")
            if no_av:
                return
            for t0 in range(ntiles):
                m = t0 - 4 * j
                c0 = max(m, 0) * P
                first = t0 == 0
                vst = v_sb[:, b, t0, :]
                if 0 <= m < 3:
                    nc.tensor.matmul(
                        psy[:, c0 : c0 + P], vst, et[:, t0, c0 : c0 + P],
                        start=first, stop=True,
                    )
                    yield True
                    nc.tensor.matmul(
                        psy[:, c0 + P : TQW], vst, et[:, t0, c0 + P : TQW],
                        start=first, stop=False,
                    )
                    yield True
                elif m == 3:
                    nc.tensor.matmul(
                        psy[:, c0:TQW], vst, et[:, t0, c0:TQW],
                        start=first, stop=True,
                    )
                    yield True
                else:
                    nc.tensor.matmul(
                        psy[:], vst, et[:, t0, :], start=first, stop=False,
                    )
                    yield True
            nc.vector.tensor_mul(
                yT_sb[:, plane, j * TQW : (j + 1) * TQW], psy[:], rcp[:]
            )
            yield True

        # ---- pipelined schedule ----
        # prologue: first A window of batch 0
        for t16 in range(4):
            a_iter(0, t16)

        c_queue = []  # ready (b, t16, co) phase-C chunks
        pending = None  # (av_gen, b, j) of the previous PJ
        plane_order = [(b, j, h) for b in range(BL) for j in range(NJ)
                       for h in range(NHL)]

        def drain_pending(push_c):
            nonlocal pending
            if pending is None:
                return
            gen, pb, pj = pending
            if gen is not None:
                for _ in gen:
                    pass
            pending = None
            if push_c:
                for t16 in range(4 * pj, 4 * pj + 4):
                    for co in range(NCO):
                        c_queue.append((pb, t16, co))

        for b, j, h in plane_order:
            av = pending[0] if pending is not None else None
            et, rcp, av = b_scores(b, j, h, av)
            if pending is not None:
                pending = (av, pending[1], pending[2])
                drain_pending(push_c=(h == 0))  # prev PJ was (b', j', h=3)
            pending = (b_av_gen(b, j, h, et, rcp), b, j)

            # filler slot: phase A iteration and/or phase C chunks
            if b == 0 and j < 3:
                a_iter(0, 4 * (j + 1) + h)
            elif b == 0 and j == 3:
                a_iter(1, h)
            elif b == 1 and j < 3:
                a_iter(1, 4 * (j + 1) + h)
            n_c = 3 if b == 0 else (4 if j < 3 else 6)
            for _ in range(min(n_c, len(c_queue))):
                c_co(*c_queue.pop(0))

        drain_pending(push_c=True)
        while c_queue:
            c_co(*c_queue.pop(0))

    nc.finalize()
    _collapse_act_table_loads(nc)
    return nc


def _collapse_act_table_loads(nc):
    """Every ACT function used here (Exp, Ln, Copy, Identity, Square) lives in
    the natural_log_exp_and_others set, but the insertion pass alternates
    exp_and_others / natural_log (one ~1.3us reload per rsqrt) — rewrite the
    first load to the combined set and drop the redundant reloads."""
    from concourse.hw_specs import get_activation_tables

    tables = list(get_activation_tables(nc.m.arch))
    target = tables.index("natural_log_exp_and_others")
    first = True
    for fn in nc.m.functions:
        for bb in fn.blocks:
            kept = []
            changed = False
            for inst in bb.instructions:
                if type(inst).__name__ == "InstLoadActFuncSet":
                    assert inst.sync_info is None
                    if first:
                        inst.act_func_set_id = target
                        first = False
                        kept.append(inst)
                    else:
                        changed = True
                    continue
                kept.append(inst)
            if changed:
                del bb.instructions[:]
                for i in kept:
                    bb.instructions.append(i)


def make_host_consts(T, dtype_bf):
    """Boundary causal mask (keep cc >= i) + PE-transpose identity."""
    i = np.arange(P)[:, None]
    cc = np.arange(P)[None, :]
    mask = (i <= cc).astype(np.float32).astype(dtype_bf)
    ident = np.eye(P, dtype=np.float32).astype(dtype_bf)
    return mask, ident


def prepare_in_maps(x, cos, sin, Wq, Wk, Wv, Wo, T=2048, C=2048, NHL=4, BL=2):
    import ml_dtypes

    bf = ml_dtypes.bfloat16
    B = x.shape[0]
    n_bgrp = B // BL
    n_hgrp = (Wq.shape[1] // HD) // NHL
    DQ = NHL * HD

    x_bf = np.ascontiguousarray(x.astype(bf))
    cosf = np.ascontiguousarray(cos.reshape(T, HD2).astype(np.float32))
    sinf = np.ascontiguousarray(sin.reshape(T, HD2).astype(np.float32))
    cs = np.ascontiguousarray(np.concatenate([cosf, sinf], axis=1))
    sc = np.ascontiguousarray(np.concatenate([sinf, cosf], axis=1))
    mask, ident = make_host_consts(T, bf)

    in_maps = []
    for g in range(n_bgrp):
        x_sh = np.ascontiguousarray(x_bf[BL * g : BL * (g + 1)].reshape(BL * T, C).T)
        for hg in range(n_hgrp):
            in_maps.append(
                {
                    "x": x_sh,
                    "wq": np.ascontiguousarray(
                        Wq[:, DQ * hg : DQ * (hg + 1)].astype(bf)
                    ),
                    "wkv": np.ascontiguousarray(
                        np.concatenate(
                            [
                                Wk[:, HD * hg : HD * (hg + 1)],
                                Wv[:, HD * hg : HD * (hg + 1)],
                            ],
                            axis=1,
                        ).astype(bf)
                    ),
                    "wo": np.ascontiguousarray(
                        Wo[DQ * hg : DQ * (hg + 1), :].astype(bf)
                    ),
                    "cs": cs,
                    "sc": sc,
                    "masks": mask,
                    "ident": ident,
                }
            )
    return in_maps


def run_on_device(x, cos, sin, Wq, Wk, Wv, Wo, trace=False):
    from concourse.bass_utils import run_bass_kernel_spmd

    T, C, NHL, BL = 2048, 2048, 4, 2
    in_maps = prepare_in_maps(x, cos, sin, Wq, Wk, Wv, Wo, T, C, NHL, BL)
    nc = build_nc(T, C, NHL, BL)
    res = run_bass_kernel_spmd(nc, in_maps, list(range(8)), trace=trace)

    B = x.shape[0]
    out = np.zeros((B, T, C), np.float32)
    n_hgrp = len(in_maps) // (B // BL)
    for g in range(B // BL):
        acc = np.zeros((BL * T, C), np.float32)
        for hg in range(n_hgrp):
            acc += res.results[g * n_hgrp + hg]["out"].astype(np.float32)
        out[BL * g : BL * (g + 1)] = acc.reshape(BL, T, C)
    return out, res


def kernel(x, cos, sin, Wq, Wk, Wv, Wo):
    out, _ = run_on_device(
        np.asarray(x), np.asarray(cos), np.asarray(sin),
        np.asarray(Wq), np.asarray(Wk), np.asarray(Wv), np.asarray(Wo),
    )
    return out


# revision 8
# speedup vs baseline: 1.1402x; 1.0213x over previous
"""Causal self-attention (GQA, RoPE, QK-RMSNorm) Trainium2 Bass kernel.

Sharding: tensor-parallel over heads x data-parallel over batch.
8 cores = 2 batch-groups (2 batches each) x 4 head-groups
(4 q heads + 1 kv head per core, GQA nrep=4).

Per core:
  - phase A: QKV projection (x^T via DMA-transpose as stationary operand),
    RoPE + RMSNorm in natural [t, d] layout, PE-transpose q,k to [d, t].
  - phase B: per (batch, j-chunk, head) "PJ": s^T = k^T.T @ q^T tiles ->
    exp on ACT (no max subtraction; rmsnormed q,k bound scores) with
    causally-trimmed widths -> boundary-block mask multiply -> E_sum
    (bf16 DVE adds) -> denominators via Pool partition_all_reduce ->
    reciprocal. AV is v-STATIONARY: psum[d, tq512] accumulates
    v[t0].T @ et[t0] over t0 (y^T lands pre-transposed, no ones column,
    no per-tqc psum juggling); per-colblock stop-flag splits keep the
    diagonal accumulation trimmed. yT_sb write folds the 1/D multiply.
  - phase C: partial out = y^T.T @ Wo_shard -> DRAM.
Host sums the 4 head-group partials per batch-group.

All matmuls bf16 inputs / fp32 PSUM accumulation; softmax, RoPE and
RMSNorm arithmetic in fp32 (E_sum accumulates in bf16; its ~0.3% rms
error on D is well inside the 2e-2 gate). rsqrt computed as
exp(-0.5*ln(x)) so every ACT call stays in one table set.
"""

import sys
from contextlib import ExitStack

import numpy as np

sys.path.insert(0, "/opt/trn_rl_repo")

import concourse.bass as bass  # noqa: E402
import concourse.bass_isa as bass_isa  # noqa: E402
import concourse.mybir as mybir  # noqa: E402
import concourse.tile as tile  # noqa: E402
from concourse import bacc  # noqa: E402

FP32 = mybir.dt.float32
BF16 = mybir.dt.bfloat16
P = 128
HD = 128
HD2 = HD // 2
TQW = 512  # tq chunk width for the QK stage


def build_nc(T=2048, C=2048, NHL=4, BL=2, repeat=1, et_bufs=2,
             xt_bufs=2, outsb_bufs=2, ps_s_bufs=3, ps_mm_bufs=2,
             drain_per_tile=1, no_qkt=0, no_av=0):
    """Build the per-core Bass program. Same program on all 8 cores."""
    TL = BL * T
    NCC = C // P  # contraction chunks
    NTB = T // P  # t-128 chunks per batch
    DQ = NHL * HD
    NJ = T // TQW  # tq-512 chunks per batch
    NK = T // P  # tk-128 chunks per batch
    NCO = C // 512
    sm_scale = float(1.0 / np.sqrt(HD))
    EPS = 1e-6

    nc = bacc.Bacc(None)
    x = nc.declare_dram_parameter("x", [C, TL], BF16, isOutput=False)
    wq = nc.declare_dram_parameter("wq", [C, DQ], BF16, isOutput=False)
    wkv = nc.declare_dram_parameter("wkv", [C, 2 * HD], BF16, isOutput=False)
    wo = nc.declare_dram_parameter("wo", [DQ, C], BF16, isOutput=False)
    cs = nc.declare_dram_parameter("cs", [T, HD], FP32, isOutput=False)
    sc = nc.declare_dram_parameter("sc", [T, HD], FP32, isOutput=False)
    masks = nc.declare_dram_parameter("masks", [P, P], BF16, isOutput=False)
    ident = nc.declare_dram_parameter("ident", [P, P], BF16, isOutput=False)
    out = nc.declare_dram_parameter("out", [TL, C], BF16, isOutput=True)

    Exp = mybir.ActivationFunctionType.Exp
    Ln = mybir.ActivationFunctionType.Ln
    xr = x.rearrange("(n p) t -> p n t", p=P)  # x is shipped pre-transposed [C, TL]

    with tile.TileContext(nc) as tc, ExitStack() as ctx:
        const = ctx.enter_context(tc.tile_pool(name="const", bufs=1))
        persist = ctx.enter_context(tc.tile_pool(name="persist", bufs=1))
        xt_pool = ctx.enter_context(tc.tile_pool(name="xt", bufs=xt_bufs))
        work = ctx.enter_context(tc.tile_pool(name="work", bufs=2))
        et_pool = ctx.enter_context(tc.tile_pool(name="et", bufs=et_bufs))
        outsb = ctx.enter_context(tc.tile_pool(name="outsb", bufs=outsb_bufs))
        ps_mm = ctx.enter_context(tc.tile_pool(name="psmm", bufs=ps_mm_bufs, space="PSUM"))
        ps_s = ctx.enter_context(tc.tile_pool(name="pss", bufs=ps_s_bufs, space="PSUM"))
        ps_y = ctx.enter_context(tc.tile_pool(name="psy", bufs=2, space="PSUM"))

        # ---- constants / persistent state ----
        wq_sb = const.tile([P, NCC, DQ], BF16)
        wqr = wq.rearrange("(n p) d -> p n d", p=P)
        nc.scalar.dma_start(wq_sb[:, 0 : NCC // 4, :], wqr[:, 0 : NCC // 4, :])
        nc.scalar.dma_start(wq_sb[:, NCC // 4 :, :], wqr[:, NCC // 4 :, :])
        wkv_sb = const.tile([P, NCC, 2 * HD], BF16)
        nc.scalar.dma_start(wkv_sb[:], wkv.rearrange("(n p) d -> p n d", p=P))
        cs_sb = const.tile([P, NTB, HD], FP32)
        nc.scalar.dma_start(cs_sb[:], cs.rearrange("(n p) d -> p n d", p=P))
        sc_sb = const.tile([P, NTB, HD], FP32)
        nc.scalar.dma_start(sc_sb[:], sc.rearrange("(n p) d -> p n d", p=P))
        ident_sb = const.tile([P, P], BF16)
        nc.scalar.dma_start(ident_sb[:], ident[:])
        mask_sb = const.tile([P, P], BF16)
        nc.scalar.dma_start(mask_sb[:], masks[:])
        ones_sb = const.tile([P, P], BF16)  # ones col [:,0:1] + row [0:1,:]
        nc.vector.memset(ones_sb[:], 1.0)
        wo_sb = const.tile([P, NHL, C], BF16)
        nc.scalar.dma_start(wo_sb[:], wo.rearrange("(h p) c -> p h c", p=P))
        eps_sb = const.tile([P, 1], FP32)
        nc.vector.memset(eps_sb[:], EPS)

        qT_sb = persist.tile([P, BL * NHL, T], BF16)  # [d, plane, t]
        kT_sb = persist.tile([P, BL, T], BF16)
        v_sb = persist.tile([P, BL, NK, HD], BF16)  # [tk_in, b, tk_chunk, d]
        yT_sb = persist.tile([P, BL * NHL, T], BF16)

        if no_qkt:
            nc.vector.memset(qT_sb[:], 0.0)
            nc.vector.memset(kT_sb[:], 0.0)
        if no_av:
            nc.vector.memset(yT_sb[:], 0.0)

        rep_ctx = tc.For_i(0, repeat, 1) if repeat > 1 else None
        if rep_ctx is not None:
            ctx.enter_context(rep_ctx)

        # ---- phase A iteration: projection + rope + rmsnorm + transpose ----
        def a_iter(b, t16):
            trow = b * T + t16 * P
            xt = xt_pool.tile([P, NCC, P], BF16, tag="xt")
            nc.sync.dma_start(xt[:], xr[:, :, trow : trow + P])

            psq = ps_mm.tile([P, 512], FP32, tag="mm512")
            pskv = ps_mm.tile([P, 512], FP32, tag="mm512")
            for c in range(NCC):
                nc.tensor.matmul(
                    psq[:, :DQ], xt[:, c, :], wq_sb[:, c, :],
                    start=(c == 0), stop=(c == NCC - 1),
                )
            for c in range(NCC):
                nc.tensor.matmul(
                    pskv[:, 0 : 2 * HD], xt[:, c, :], wkv_sb[:, c, :],
                    start=(c == 0), stop=(c == NCC - 1),
                )

            # rope: p1 = [x1*cos | x2*sin], p2 = [x1*sin | x2*cos]
            # y1 = p1_lo + p1_hi ; y2 = p2_hi - p2_lo
            psq3 = psq[:, 0:DQ].rearrange("p (h d) -> p h d", d=HD)
            csb = cs_sb[:, t16, None, :].to_broadcast((P, NHL, HD))
            scb = sc_sb[:, t16, None, :].to_broadcast((P, NHL, HD))
            p1 = work.tile([P, NHL, HD], FP32, tag="p1")
            p2 = work.tile([P, NHL, HD], FP32, tag="p2")
            nc.vector.tensor_mul(p1[:], psq3, csb)
            nc.vector.tensor_mul(p2[:], psq3, scb)
            q_ro = work.tile([P, NHL, HD], FP32, tag="qro")
            nc.vector.tensor_add(
                q_ro[:, :, 0:HD2], p1[:, :, 0:HD2], p1[:, :, HD2:HD]
            )
            nc.vector.tensor_sub(
                q_ro[:, :, HD2:HD], p2[:, :, HD2:HD], p2[:, :, 0:HD2]
            )
            p1k = work.tile([P, HD], FP32, tag="p1k")
            p2k = work.tile([P, HD], FP32, tag="p2k")
            nc.vector.tensor_mul(p1k[:], pskv[:, 0:HD], cs_sb[:, t16, :])
            nc.vector.tensor_mul(p2k[:], pskv[:, 0:HD], sc_sb[:, t16, :])
            k_ro = work.tile([P, HD], FP32, tag="kro")
            nc.vector.tensor_add(k_ro[:, 0:HD2], p1k[:, 0:HD2], p1k[:, HD2:HD])
            nc.vector.tensor_sub(k_ro[:, HD2:HD], p2k[:, HD2:HD], p2k[:, 0:HD2])

            # rmsnorm scales: rs = exp(-0.5 * ln(ssq/HD + eps))
            sq = work.tile([P, NHL, HD], FP32, tag="p1")  # reuse p1 ring (SBUF)
            nc.vector.tensor_mul(sq[:], q_ro[:], q_ro[:])
            sqk = work.tile([P, HD], FP32, tag="sqk")
            nc.vector.tensor_mul(sqk[:], k_ro[:], k_ro[:])
            ssq = work.tile([P, NHL + 1], FP32, tag="ssq")
            nc.vector.tensor_reduce(
                ssq[:, 0:NHL], sq[:], axis=mybir.AxisListType.X,
                op=mybir.AluOpType.add,
            )
            nc.vector.tensor_reduce(
                ssq[:, NHL : NHL + 1], sqk[:], axis=mybir.AxisListType.X,
                op=mybir.AluOpType.add,
            )
            lnv = work.tile([P, NHL + 1], FP32, tag="lnv")
            nc.scalar.activation(
                lnv[:], ssq[:], Ln, bias=eps_sb[:, 0:1], scale=1.0 / HD
            )
            rs = work.tile([P, NHL + 1], FP32, tag="rs")
            nc.scalar.activation(rs[:], lnv[:], Exp, scale=-0.5)

            q_n = work.tile([P, NHL, HD], BF16, tag="qn")
            nc.vector.tensor_mul(
                q_n[:], q_ro[:], rs[:, 0:NHL, None].to_broadcast((P, NHL, HD))
            )
            k_n = work.tile([P, HD], BF16, tag="kn")
            nc.vector.tensor_scalar_mul(k_n[:], k_ro[:], rs[:, NHL : NHL + 1])

            nc.vector.tensor_copy(v_sb[:, b, t16, 0:HD], pskv[:, HD : 2 * HD])

            if no_qkt:
                return
            pt = ps_y.tile([P, NHL + 1, 132], BF16, tag="ptq", bufs=1)
            for h in range(NHL):
                nc.tensor.transpose(pt[:, h, 0:P], q_n[:, h, :], ident_sb[:])
            nc.tensor.transpose(pt[:, NHL, 0:P], k_n[:], ident_sb[:])
            nc.any.tensor_copy(
                qT_sb[:, b * NHL : b * NHL + NHL, t16 * P : (t16 + 1) * P],
                pt[:, 0:NHL, 0:P],
            )
            nc.any.tensor_copy(
                kT_sb[:, b, t16 * P : (t16 + 1) * P], pt[:, NHL, 0:P]
            )

        # ---- phase C: one output-column chunk of Wo for one (b, t16) ----
        def c_co(b, t16, co):
            pso = ps_mm.tile([P, 512], FP32, tag="mm512", name=f"pso{co}")
            for h in range(NHL):
                nc.tensor.matmul(
                    pso[:],
                    yT_sb[:, b * NHL + h, t16 * P : (t16 + 1) * P],
                    wo_sb[:, h, co * 512 : (co + 1) * 512],
                    start=(h == 0), stop=(h == NHL - 1),
                )
            o_sb = outsb.tile([P, 512], BF16, tag="osb")
            nc.any.tensor_copy(o_sb[:], pso[:])
            trow = b * T + t16 * P
            nc.sync.dma_start(
                out[trow : trow + P, co * 512 : (co + 1) * 512], o_sb[:]
            )

        # ---- phase B: scores+exp+denominator for one PJ=(b,j,h) ----
        def b_scores(b, j, h, av_gen):
            """Emit score matmuls + exp + mask + E_sum for PJ (b,j,h),
            interleaving the pending AV matmul stream (av_gen) of the
            previous PJ so PE has work while ACT chews through the exps.
            Ends with the denominator chain (Pool allreduce + reciprocal).
            Returns (et_slot, rcp_tile, av_gen)."""
            plane = b * NHL + h
            ntiles = 4 * j + 4
            et = et_pool.tile([P, 4 * NJ, TQW], BF16, tag="et")
            es = work.tile([P, TQW], BF16, tag="esum")
            for t0 in range(ntiles):
                m = t0 - 4 * j
                c0 = max(m, 0) * P
                ps = ps_s.tile([P, TQW], FP32, tag="s")
                nc.tensor.matmul(
                    ps[:, c0:TQW],
                    kT_sb[:, b, t0 * P : (t0 + 1) * P],
                    qT_sb[:, plane, j * TQW + c0 : (j + 1) * TQW],
                    start=True, stop=True,
                )
                nc.scalar.activation(
                    et[:, t0, c0:TQW], ps[:, c0:TQW], Exp, scale=sm_scale
                )
                if m >= 0:
                    nc.vector.tensor_mul(
                        et[:, t0, c0 : c0 + P], et[:, t0, c0 : c0 + P],
                        mask_sb[:],
                    )
                if t0 == 0:
                    nc.vector.tensor_copy(es[:], et[:, 0, :])
                else:
                    nc.vector.tensor_add(
                        es[:, c0:TQW], es[:, c0:TQW], et[:, t0, c0:TQW]
                    )
                if av_gen is not None:
                    for _ in range(drain_per_tile):
                        if next(av_gen, None) is None:
                            av_gen = None
                            break
            # denominator: ONE all-ones [128,128] stationary matmul both
            # collapses the partition dim and broadcasts: out[m,n] = D[n]
            # for every m. (gpsimd allreduce is ~5us/call on HW; a PE->ACT
            # ->PE chain stalls the in-order ACT queue.)
            bc_ps = ps_s.tile([P, TQW], FP32, tag="s", name="bcps")
            nc.tensor.matmul(
                bc_ps[:], ones_sb[:], es[:], start=True, stop=True
            )
            rcp = work.tile([P, TQW], FP32, tag="rcp")
            nc.vector.reciprocal(rcp[:], bc_ps[:])
            return et, rcp, av_gen

        def b_av_gen(b, j, h, et, rcp):
            """v-stationary AV: psum[d, tq512] accumulates v[t0].T @ et[t0]
            with causal trimming; diagonal tiles split per colblock so the
            stop flags close each column range exactly once. Ends with the
            1/D-folded copy into yT_sb. Yields after each matmul."""
            plane = b * NHL + h
            ntiles = 4 * j + 4
            psy = ps_y.tile([P, TQW], FP32, tag="yt")
            if no_av:
                return
            for t0 in range(ntiles):
                m = t0 - 4 * j
                c0 = max(m, 0) * P
                first = t0 == 0
                vst = v_sb[:, b, t0, :]
                if 0 <= m < 3:
                    nc.tensor.matmul(
                        psy[:, c0 : c0 + P], vst, et[:, t0, c0 : c0 + P],
                        start=first, stop=True,
                    )
                    yield True
                    nc.tensor.matmul(
                        psy[:, c0 + P : TQW], vst, et[:, t0, c0 + P : TQW],
                        start=first, stop=False,
                    )
                    yield True
                elif m == 3:
                    nc.tensor.matmul(
                        psy[:, c0:TQW], vst, et[:, t0, c0:TQW],
                        start=first, stop=True,
                    )
                    yield True
                else:
                    nc.tensor.matmul(
                        psy[:], vst, et[:, t0, :], start=first, stop=False,
                    )
                    yield True
            nc.vector.tensor_mul(
                yT_sb[:, plane, j * TQW : (j + 1) * TQW], psy[:], rcp[:]
            )
            yield True

        # ---- pipelined schedule ----
        # prologue: first A window of batch 0
        for t16 in range(4):
            a_iter(0, t16)

        c_queue = []  # ready (b, t16, co) phase-C chunks
        pending = None  # (av_gen, b, j) of the previous PJ
        plane_order = [(b, j, h) for b in range(BL) for j in range(NJ)
                       for h in range(NHL)]

        def drain_pending(push_c):
            nonlocal pending
            if pending is None:
                return
            gen, pb, pj = pending
            if gen is not None:
                for _ in gen:
                    pass
            pending = None
            if push_c:
                for t16 in range(4 * pj, 4 * pj + 4):
                    for co in range(NCO):
                        c_queue.append((pb, t16, co))

        for b, j, h in plane_order:
            av = pending[0] if pending is not None else None
            et, rcp, av = b_scores(b, j, h, av)
            if pending is not None:
                pending = (av, pending[1], pending[2])
                drain_pending(push_c=(h == 0))  # prev PJ was (b', j', h=3)
            pending = (b_av_gen(b, j, h, et, rcp), b, j)

            # filler slot: phase A iteration and/or phase C chunks
            if b == 0 and j < 3:
                a_iter(0, 4 * (j + 1) + h)
            elif b == 0 and j == 3:
                a_iter(1, h)
            elif b == 1 and j < 3:
                a_iter(1, 4 * (j + 1) + h)
            n_c = 3 if b == 0 else (4 if j < 3 else 6)
            for _ in range(min(n_c, len(c_queue))):
                c_co(*c_queue.pop(0))

        drain_pending(push_c=True)
        while c_queue:
            c_co(*c_queue.pop(0))

    nc.finalize()
    _collapse_act_table_loads(nc)
    return nc


def _collapse_act_table_loads(nc):
    """Every ACT function used here (Exp, Ln, Copy, Identity, Square) lives in
    the natural_log_exp_and_others set, but the insertion pass alternates
    exp_and_others / natural_log (one ~1.3us reload per rsqrt) — rewrite the
    first load to the combined set and drop the redundant reloads."""
    from concourse.hw_specs import get_activation_tables

    tables = list(get_activation_tables(nc.m.arch))
    target = tables.index("natural_log_exp_and_others")
    first = True
    for fn in nc.m.functions:
        for bb in fn.blocks:
            kept = []
            changed = False
            for inst in bb.instructions:
                if type(inst).__name__ == "InstLoadActFuncSet":
                    assert inst.sync_info is None
                    if first:
                        inst.act_func_set_id = target
                        first = False
                        kept.append(inst)
                    else:
                        changed = True
                    continue
                kept.append(inst)
            if changed:
                del bb.instructions[:]
                for i in kept:
                    bb.instructions.append(i)


def make_host_consts(T, dtype_bf):
    """Boundary causal mask (keep cc >= i) + PE-transpose identity."""
    i = np.arange(P)[:, None]
    cc = np.arange(P)[None, :]
    mask = (i <= cc).astype(np.float32).astype(dtype_bf)
    ident = np.eye(P, dtype=np.float32).astype(dtype_bf)
    return mask, ident


def prepare_in_maps(x, cos, sin, Wq, Wk, Wv, Wo, T=2048, C=2048, NHL=4, BL=2):
    import ml_dtypes

    bf = ml_dtypes.bfloat16
    B = x.shape[0]
    n_bgrp = B // BL
    n_hgrp = (Wq.shape[1] // HD) // NHL
    DQ = NHL * HD

    x_bf = np.ascontiguousarray(x.astype(bf))
    cosf = np.ascontiguousarray(cos.reshape(T, HD2).astype(np.float32))
    sinf = np.ascontiguousarray(sin.reshape(T, HD2).astype(np.float32))
    cs = np.ascontiguousarray(np.concatenate([cosf, sinf], axis=1))
    sc = np.ascontiguousarray(np.concatenate([sinf, cosf], axis=1))
    mask, ident = make_host_consts(T, bf)

    in_maps = []
    for g in range(n_bgrp):
        x_sh = np.ascontiguousarray(x_bf[BL * g : BL * (g + 1)].reshape(BL * T, C).T)
        for hg in range(n_hgrp):
            in_maps.append(
                {
                    "x": x_sh,
                    "wq": np.ascontiguousarray(
                        Wq[:, DQ * hg : DQ * (hg + 1)].astype(bf)
                    ),
                    "wkv": np.ascontiguousarray(
                        np.concatenate(
                            [
                                Wk[:, HD * hg : HD * (hg + 1)],
                                Wv[:, HD * hg : HD * (hg + 1)],
                            ],
                            axis=1,
                        ).astype(bf)
                    ),
                    "wo": np.ascontiguousarray(
                        Wo[DQ * hg : DQ * (hg + 1), :].astype(bf)
                    ),
                    "cs": cs,
                    "sc": sc,
                    "masks": mask,
                    "ident": ident,
                }
            )
    return in_maps


def run_on_device(x, cos, sin, Wq, Wk, Wv, Wo, trace=False):
    from concourse.bass_utils import run_bass_kernel_spmd

    T, C, NHL, BL = 2048, 2048, 4, 2
    in_maps = prepare_in_maps(x, cos, sin, Wq, Wk, Wv, Wo, T, C, NHL, BL)
    nc = build_nc(T, C, NHL, BL)
    res = run_bass_kernel_spmd(nc, in_maps, list(range(8)), trace=trace)

    B = x.shape[0]
    out = np.zeros((B, T, C), np.float32)
    n_hgrp = len(in_maps) // (B // BL)
    for g in range(B // BL):
        acc = np.zeros((BL * T, C), np.float32)
        for hg in range(n_hgrp):
            acc += res.results[g * n_hgrp + hg]["out"].astype(np.float32)
        out[BL * g : BL * (g + 1)] = acc.reshape(BL, T, C)
    return out, res


def kernel(x, cos, sin, Wq, Wk, Wv, Wo):
    out, _ = run_on_device(
        np.asarray(x), np.asarray(cos), np.asarray(sin),
        np.asarray(Wq), np.asarray(Wk), np.asarray(Wv), np.asarray(Wo),
    )
    return out


# revision 29
# speedup vs baseline: 1.2627x; 1.1074x over previous
"""Causal self-attention (GQA, RoPE, QK-RMSNorm) Trainium2 Bass kernel.

Sharding: tensor-parallel over heads x data-parallel over batch.
8 cores = 2 batch-groups (2 batches each) x 4 head-groups
(4 q heads + 1 kv head per core, GQA nrep=4).

Per core:
  - phase A: QKV projection (x^T via DMA-transpose as stationary operand),
    RoPE + RMSNorm in natural [t, d] layout, PE-transpose q,k to [d, t].
  - phase B: per (batch, j-chunk, head) "PJ": s^T = k^T.T @ q^T tiles ->
    exp on ACT (no max subtraction; rmsnormed q,k bound scores) with
    causally-trimmed widths -> boundary-block mask multiply -> E_sum
    (bf16 DVE adds) -> denominators via Pool partition_all_reduce ->
    reciprocal. AV is v-STATIONARY: psum[d, tq512] accumulates
    v[t0].T @ et[t0] over t0 (y^T lands pre-transposed, no ones column,
    no per-tqc psum juggling); per-colblock stop-flag splits keep the
    diagonal accumulation trimmed. yT_sb write folds the 1/D multiply.
  - phase C: partial out = y^T.T @ Wo_shard -> DRAM.
Host sums the 4 head-group partials per batch-group.

All matmuls bf16 inputs / fp32 PSUM accumulation; softmax, RoPE and
RMSNorm arithmetic in fp32 (E_sum accumulates in bf16; its ~0.3% rms
error on D is well inside the 2e-2 gate). rsqrt computed as
exp(-0.5*ln(x)) so every ACT call stays in one table set.
"""

import sys
from contextlib import ExitStack

import numpy as np

sys.path.insert(0, "/opt/trn_rl_repo")

import concourse.bass as bass  # noqa: E402
import concourse.bass_isa as bass_isa  # noqa: E402
import concourse.mybir as mybir  # noqa: E402
import concourse.tile as tile  # noqa: E402
from concourse import bacc  # noqa: E402

FP32 = mybir.dt.float32
BF16 = mybir.dt.bfloat16
P = 128
HD = 128
HD2 = HD // 2
TQW = 512  # tq chunk width for the QK stage


def build_nc(T=2048, C=2048, NHL=4, BL=2, repeat=1, et_bufs=2,
             xt_bufs=2, outsb_bufs=2, ps_s_bufs=3, ps_mm_bufs=2,
             drain_per_tile=1, no_qkt=0, no_av=0, no_esum=0, no_dchain=0,
             no_mask=0):
    """Build the per-core Bass program. Same program on all 8 cores."""
    TL = BL * T
    NCC = C // P  # contraction chunks
    NTB = T // P  # t-128 chunks per batch
    DQ = NHL * HD
    NJ = T // TQW  # tq-512 chunks per batch
    NK = T // P  # tk-128 chunks per batch
    NCO = C // 512
    sm_scale = float(1.0 / np.sqrt(HD))
    EPS = 1e-6

    nc = bacc.Bacc(None)
    x = nc.declare_dram_parameter("x", [C, TL], BF16, isOutput=False)
    wq = nc.declare_dram_parameter("wq", [C, DQ], BF16, isOutput=False)
    wkv = nc.declare_dram_parameter("wkv", [C, 2 * HD], BF16, isOutput=False)
    wo = nc.declare_dram_parameter("wo", [DQ, C], BF16, isOutput=False)
    cs = nc.declare_dram_parameter("cs", [T, HD], FP32, isOutput=False)
    sc = nc.declare_dram_parameter("sc", [T, HD], FP32, isOutput=False)
    masks = nc.declare_dram_parameter("masks", [P, P], BF16, isOutput=False)
    ident = nc.declare_dram_parameter("ident", [P, P], BF16, isOutput=False)
    out = nc.declare_dram_parameter("out", [TL, C], BF16, isOutput=True)

    Exp = mybir.ActivationFunctionType.Exp
    Ln = mybir.ActivationFunctionType.Ln
    xr = x.rearrange("(n p) t -> p n t", p=P)  # x is shipped pre-transposed [C, TL]

    with tile.TileContext(nc) as tc, ExitStack() as ctx:
        const = ctx.enter_context(tc.tile_pool(name="const", bufs=1))
        persist = ctx.enter_context(tc.tile_pool(name="persist", bufs=1))
        xt_pool = ctx.enter_context(tc.tile_pool(name="xt", bufs=xt_bufs))
        work = ctx.enter_context(tc.tile_pool(name="work", bufs=2))
        et_pool = ctx.enter_context(tc.tile_pool(name="et", bufs=et_bufs))
        outsb = ctx.enter_context(tc.tile_pool(name="outsb", bufs=outsb_bufs))
        ps_mm = ctx.enter_context(tc.tile_pool(name="psmm", bufs=ps_mm_bufs, space="PSUM"))
        ps_s = ctx.enter_context(tc.tile_pool(name="pss", bufs=ps_s_bufs, space="PSUM"))
        ps_y = ctx.enter_context(tc.tile_pool(name="psy", bufs=2, space="PSUM"))

        # ---- constants / persistent state ----
        wq_sb = const.tile([P, NCC, DQ], BF16)
        wqr = wq.rearrange("(n p) d -> p n d", p=P)
        nc.scalar.dma_start(wq_sb[:, 0 : NCC // 4, :], wqr[:, 0 : NCC // 4, :])
        nc.scalar.dma_start(wq_sb[:, NCC // 4 :, :], wqr[:, NCC // 4 :, :])
        wkv_sb = const.tile([P, NCC, 2 * HD], BF16)
        nc.scalar.dma_start(wkv_sb[:], wkv.rearrange("(n p) d -> p n d", p=P))
        cs_sb = const.tile([P, NTB, HD], FP32)
        nc.scalar.dma_start(cs_sb[:], cs.rearrange("(n p) d -> p n d", p=P))
        sc_sb = const.tile([P, NTB, HD], FP32)
        nc.scalar.dma_start(sc_sb[:], sc.rearrange("(n p) d -> p n d", p=P))
        ident_sb = const.tile([P, P], BF16)
        nc.scalar.dma_start(ident_sb[:], ident[:])
        mask_sb = const.tile([P, P], BF16)
        nc.scalar.dma_start(mask_sb[:], masks[:])
        ones_sb = const.tile([P, P], BF16)  # ones col [:,0:1] + row [0:1,:]
        nc.vector.memset(ones_sb[:], 1.0)
        wo_sb = const.tile([P, NHL, C], BF16)
        nc.scalar.dma_start(wo_sb[:], wo.rearrange("(h p) c -> p h c", p=P))
        eps_sb = const.tile([P, 1], FP32)
        nc.vector.memset(eps_sb[:], EPS)

        qT_sb = persist.tile([P, BL * NHL, T], BF16)  # [d, plane, t]
        kT_sb = persist.tile([P, BL, T], BF16)
        v_sb = persist.tile([P, BL, NK, HD], BF16)  # [tk_in, b, tk_chunk, d]
        yT_sb = persist.tile([P, BL * NHL, T], BF16)

        if no_qkt:
            nc.vector.memset(qT_sb[:], 0.0)
            nc.vector.memset(kT_sb[:], 0.0)
        if no_av:
            nc.vector.memset(yT_sb[:], 0.0)
        rcp_const = None
        if no_dchain:
            rcp_const = persist.tile([P, TQW], FP32)
            nc.vector.memset(rcp_const[:], 1.0)
        if no_esum:
            es_init = persist.tile([P, TQW], BF16)
            nc.vector.memset(es_init[:], 1.0)

        rep_ctx = tc.For_i(0, repeat, 1) if repeat > 1 else None
        if rep_ctx is not None:
            ctx.enter_context(rep_ctx)

        # ---- phase A iteration: projection + rope + rmsnorm + transpose ----
        def a_iter(b, t16):
            trow = b * T + t16 * P
            xt = xt_pool.tile([P, NCC, P], BF16, tag="xt")
            nc.sync.dma_start(xt[:], xr[:, :, trow : trow + P])

            psq = ps_mm.tile([P, 512], FP32, tag="mm512")
            pskv = ps_mm.tile([P, 512], FP32, tag="mm512")
            for c in range(NCC):
                nc.tensor.matmul(
                    psq[:, :DQ], xt[:, c, :], wq_sb[:, c, :],
                    start=(c == 0), stop=(c == NCC - 1),
                )
            for c in range(NCC):
                nc.tensor.matmul(
                    pskv[:, 0 : 2 * HD], xt[:, c, :], wkv_sb[:, c, :],
                    start=(c == 0), stop=(c == NCC - 1),
                )

            # rope: p1 = [x1*cos | x2*sin], p2 = [x1*sin | x2*cos]
            # y1 = p1_lo + p1_hi ; y2 = p2_hi - p2_lo
            psq3 = psq[:, 0:DQ].rearrange("p (h d) -> p h d", d=HD)
            csb = cs_sb[:, t16, None, :].to_broadcast((P, NHL, HD))
            scb = sc_sb[:, t16, None, :].to_broadcast((P, NHL, HD))
            p1 = work.tile([P, NHL, HD], FP32, tag="p1")
            p2 = work.tile([P, NHL, HD], FP32, tag="p2")
            nc.vector.tensor_mul(p1[:], psq3, csb)
            nc.vector.tensor_mul(p2[:], psq3, scb)
            q_ro = work.tile([P, NHL, HD], FP32, tag="qro")
            nc.vector.tensor_add(
                q_ro[:, :, 0:HD2], p1[:, :, 0:HD2], p1[:, :, HD2:HD]
            )
            nc.vector.tensor_sub(
                q_ro[:, :, HD2:HD], p2[:, :, HD2:HD], p2[:, :, 0:HD2]
            )
            p1k = work.tile([P, HD], FP32, tag="p1k")
            p2k = work.tile([P, HD], FP32, tag="p2k")
            nc.vector.tensor_mul(p1k[:], pskv[:, 0:HD], cs_sb[:, t16, :])
            nc.vector.tensor_mul(p2k[:], pskv[:, 0:HD], sc_sb[:, t16, :])
            k_ro = work.tile([P, HD], FP32, tag="kro")
            nc.vector.tensor_add(k_ro[:, 0:HD2], p1k[:, 0:HD2], p1k[:, HD2:HD])
            nc.vector.tensor_sub(k_ro[:, HD2:HD], p2k[:, HD2:HD], p2k[:, 0:HD2])

            # rmsnorm scales: rs = exp(-0.5 * ln(ssq/HD + eps))
            sq = work.tile([P, NHL, HD], FP32, tag="p1")  # reuse p1 ring (SBUF)
            nc.vector.tensor_mul(sq[:], q_ro[:], q_ro[:])
            sqk = work.tile([P, HD], FP32, tag="sqk")
            nc.vector.tensor_mul(sqk[:], k_ro[:], k_ro[:])
            ssq = work.tile([P, NHL + 1], FP32, tag="ssq")
            nc.vector.tensor_reduce(
                ssq[:, 0:NHL], sq[:], axis=mybir.AxisListType.X,
                op=mybir.AluOpType.add,
            )
            nc.vector.tensor_reduce(
                ssq[:, NHL : NHL + 1], sqk[:], axis=mybir.AxisListType.X,
                op=mybir.AluOpType.add,
            )
            lnv = work.tile([P, NHL + 1], FP32, tag="lnv")
            nc.scalar.activation(
                lnv[:], ssq[:], Ln, bias=eps_sb[:, 0:1], scale=1.0 / HD
            )
            rs = work.tile([P, NHL + 1], FP32, tag="rs")
            nc.scalar.activation(rs[:], lnv[:], Exp, scale=-0.5)

            q_n = work.tile([P, NHL, HD], BF16, tag="qn")
            nc.vector.tensor_mul(
                q_n[:], q_ro[:], rs[:, 0:NHL, None].to_broadcast((P, NHL, HD))
            )
            k_n = work.tile([P, HD], BF16, tag="kn")
            nc.vector.tensor_scalar_mul(k_n[:], k_ro[:], rs[:, NHL : NHL + 1])

            nc.vector.tensor_copy(v_sb[:, b, t16, 0:HD], pskv[:, HD : 2 * HD])

            if no_qkt:
                return
            pt = ps_y.tile([P, NHL + 1, 132], BF16, tag="ptq", bufs=1)
            for h in range(NHL):
                nc.tensor.transpose(pt[:, h, 0:P], q_n[:, h, :], ident_sb[:])
            nc.tensor.transpose(pt[:, NHL, 0:P], k_n[:], ident_sb[:])
            nc.any.tensor_copy(
                qT_sb[:, b * NHL : b * NHL + NHL, t16 * P : (t16 + 1) * P],
                pt[:, 0:NHL, 0:P],
            )
            nc.any.tensor_copy(
                kT_sb[:, b, t16 * P : (t16 + 1) * P], pt[:, NHL, 0:P]
            )

        # ---- phase C: one output-column chunk of Wo for one (b, t16) ----
        def c_co(b, t16, co):
            pso = ps_mm.tile([P, 512], FP32, tag="mm512", name=f"pso{co}")
            for h in range(NHL):
                nc.tensor.matmul(
                    pso[:],
                    yT_sb[:, b * NHL + h, t16 * P : (t16 + 1) * P],
                    wo_sb[:, h, co * 512 : (co + 1) * 512],
                    start=(h == 0), stop=(h == NHL - 1),
                )
            o_sb = outsb.tile([P, 512], BF16, tag="osb")
            nc.scalar.activation(
                o_sb[:], pso[:], mybir.ActivationFunctionType.Copy
            )
            trow = b * T + t16 * P
            nc.sync.dma_start(
                out[trow : trow + P, co * 512 : (co + 1) * 512], o_sb[:]
            )

        # ---- phase B: scores+exp+denominator for one PJ=(b,j,h) ----
        def b_scores(b, j, h, av_gen):
            """Emit score matmuls + exp + mask + E_sum for PJ (b,j,h),
            interleaving the pending AV matmul stream (av_gen) of the
            previous PJ so PE has work while ACT chews through the exps.
            Returns (et_slot, es_tile, av_gen)."""
            plane = b * NHL + h
            ntiles = 4 * j + 4
            et = et_pool.tile([P, 4 * NJ, TQW], BF16, tag="et")
            es = es_init if no_esum else work.tile([P, TQW], BF16, tag="esum")
            for t0 in range(ntiles):
                m = t0 - 4 * j
                c0 = max(m, 0) * P
                ps = ps_s.tile([P, TQW], FP32, tag="s")
                nc.tensor.matmul(
                    ps[:, c0:TQW],
                    kT_sb[:, b, t0 * P : (t0 + 1) * P],
                    qT_sb[:, plane, j * TQW + c0 : (j + 1) * TQW],
                    start=True, stop=True,
                )
                nc.scalar.activation(
                    et[:, t0, c0:TQW], ps[:, c0:TQW], Exp, scale=sm_scale
                )
                if m >= 0 and not no_mask:
                    nc.vector.tensor_mul(
                        et[:, t0, c0 : c0 + P], et[:, t0, c0 : c0 + P],
                        mask_sb[:],
                    )
                if not no_esum:
                    if t0 == 0:
                        nc.vector.tensor_copy(es[:], et[:, 0, :])
                    else:
                        nc.vector.tensor_add(
                            es[:, c0:TQW], es[:, c0:TQW], et[:, t0, c0:TQW]
                        )
                if av_gen is not None:
                    n_drain = drain_per_tile + (1 if t0 >= ntiles - 5 else 0)
                    for _ in range(n_drain):
                        if next(av_gen, None) is None:
                            av_gen = None
                            break
            return et, es, av_gen

        def b_av_gen(b, j, h, et, es):
            """v-stationary AV: psum[d, tq512] accumulates v[t0].T @ et[t0]
            with causal trimming; diagonal tiles split per colblock so the
            stop flags close each column range exactly once. The denominator
            chain is emitted at the END of this stream (one PJ later in the
            PE queue) so the in-order PE never stalls on the exp/mask/es
            tail: ONE all-ones [128,128] stationary matmul both collapses
            the partition dim and broadcasts (out[m,n] = D[n] for every m),
            then reciprocal, then the 1/D-folded copy into yT_sb. (A gpsimd
            allreduce is ~5us/call on HW; a PE->ACT->PE chain stalls the
            in-order ACT queue.)"""
            plane = b * NHL + h
            ntiles = 4 * j + 4
            psy = ps_y.tile([P, TQW], FP32, tag="yt")
            if no_av:
                return
            rcp = rcp_const
            for t0 in range(ntiles):
                m = t0 - 4 * j
                c0 = max(m, 0) * P
                first = t0 == 0
                vst = v_sb[:, b, t0, :]
                if 0 <= m < 3:
                    nc.tensor.matmul(
                        psy[:, c0 : c0 + P], vst, et[:, t0, c0 : c0 + P],
                        start=first, stop=True,
                    )
                    yield True
                    nc.tensor.matmul(
                        psy[:, c0 + P : TQW], vst, et[:, t0, c0 + P : TQW],
                        start=first, stop=False,
                    )
                    yield True
                elif m == 3:
                    nc.tensor.matmul(
                        psy[:, c0:TQW], vst, et[:, t0, c0:TQW],
                        start=first, stop=True,
                    )
                    yield True
                else:
                    nc.tensor.matmul(
                        psy[:], vst, et[:, t0, :], start=first, stop=False,
                    )
                    yield True
            if not no_dchain:
                bc_ps = ps_s.tile([P, TQW], FP32, tag="s", name="bcps")
                nc.tensor.matmul(
                    bc_ps[:], ones_sb[:], es[:], start=True, stop=True
                )
                # 1/D as exp(-ln(D)) on ACT: DVE reciprocal of [128,512] is
                # multi-pass on HW (~1.2us/PJ); ACT has slack and its Ln
                # read frees the borrowed ps_s slot quickly.
                lnd = work.tile([P, TQW], FP32, tag="lnd")
                nc.scalar.activation(lnd[:], bc_ps[:], Ln)
                rcp = work.tile([P, TQW], FP32, tag="rcp")
                nc.scalar.activation(rcp[:], lnd[:], Exp, scale=-1.0)
                yield True
            nc.vector.tensor_mul(
                yT_sb[:, plane, j * TQW : (j + 1) * TQW], psy[:], rcp[:]
            )
            yield True

        # ---- pipelined schedule ----
        # prologue: first A window of batch 0
        for t16 in range(4):
            a_iter(0, t16)

        c_queue = []  # ready (b, t16, co) phase-C chunks
        c_staged = []  # chunks whose yT just landed; promoted next iter
        pending = None  # (av_gen, b, j) of the previous PJ
        plane_order = [(b, j, h) for b in range(BL) for j in range(NJ)
                       for h in range(NHL)]

        def drain_pending(push_c):
            nonlocal pending
            if pending is None:
                return
            gen, pb, pj = pending
            if gen is not None:
                for _ in gen:
                    pass
            pending = None
            if push_c:
                for t16 in range(4 * pj, 4 * pj + 4):
                    for co in range(NCO):
                        c_staged.append((pb, t16, co))

        for b, j, h in plane_order:
            c_queue.extend(c_staged)
            del c_staged[:]
            av = pending[0] if pending is not None else None
            et, es, av = b_scores(b, j, h, av)
            if pending is not None:
                pending = (av, pending[1], pending[2])
                drain_pending(push_c=(h == 0))  # prev PJ was (b', j', h=3)
            pending = (b_av_gen(b, j, h, et, es), b, j)

            # filler slot: phase A iteration and/or phase C chunks
            if b == 0 and j < 3:
                a_iter(0, 4 * (j + 1) + h)
            elif b == 0 and j == 3:
                a_iter(1, h)
            elif b == 1 and j < 3:
                a_iter(1, 4 * (j + 1) + h)
            n_c = 3 if b == 0 else (4 if j < 3 else 6)
            for _ in range(min(n_c, len(c_queue))):
                c_co(*c_queue.pop(0))

        drain_pending(push_c=True)
        c_queue.extend(c_staged)
        del c_staged[:]
        while c_queue:
            c_co(*c_queue.pop(0))

    nc.finalize()
    _collapse_act_table_loads(nc)
    return nc


def _collapse_act_table_loads(nc):
    """Every ACT function used here (Exp, Ln, Copy, Identity, Square) lives in
    the natural_log_exp_and_others set, but the insertion pass alternates
    exp_and_others / natural_log (one ~1.3us reload per rsqrt) — rewrite the
    first load to the combined set and drop the redundant reloads."""
    from concourse.hw_specs import get_activation_tables

    tables = list(get_activation_tables(nc.m.arch))
    target = tables.index("natural_log_exp_and_others")
    first = True
    for fn in nc.m.functions:
        for bb in fn.blocks:
            kept = []
            changed = False
            for inst in bb.instructions:
                if type(inst).__name__ == "InstLoadActFuncSet":
                    assert inst.sync_info is None
                    if first:
                        inst.act_func_set_id = target
                        first = False
                        kept.append(inst)
                    else:
                        changed = True
                    continue
                kept.append(inst)
            if changed:
                del bb.instructions[:]
                for i in kept:
                    bb.instructions.append(i)


def make_host_consts(T, dtype_bf):
    """Boundary causal mask (keep cc >= i) + PE-transpose identity."""
    i = np.arange(P)[:, None]
    cc = np.arange(P)[None, :]
    mask = (i <= cc).astype(np.float32).astype(dtype_bf)
    ident = np.eye(P, dtype=np.float32).astype(dtype_bf)
    return mask, ident


def prepare_in_maps(x, cos, sin, Wq, Wk, Wv, Wo, T=2048, C=2048, NHL=4, BL=2):
    import ml_dtypes

    bf = ml_dtypes.bfloat16
    B = x.shape[0]
    n_bgrp = B // BL
    n_hgrp = (Wq.shape[1] // HD) // NHL
    DQ = NHL * HD

    x_bf = np.ascontiguousarray(x.astype(bf))
    cosf = np.ascontiguousarray(cos.reshape(T, HD2).astype(np.float32))
    sinf = np.ascontiguousarray(sin.reshape(T, HD2).astype(np.float32))
    cs = np.ascontiguousarray(np.concatenate([cosf, sinf], axis=1))
    sc = np.ascontiguousarray(np.concatenate([sinf, cosf], axis=1))
    mask, ident = make_host_consts(T, bf)

    in_maps = []
    for g in range(n_bgrp):
        x_sh = np.ascontiguousarray(x_bf[BL * g : BL * (g + 1)].reshape(BL * T, C).T)
        for hg in range(n_hgrp):
            in_maps.append(
                {
                    "x": x_sh,
                    "wq": np.ascontiguousarray(
                        Wq[:, DQ * hg : DQ * (hg + 1)].astype(bf)
                    ),
                    "wkv": np.ascontiguousarray(
                        np.concatenate(
                            [
                                Wk[:, HD * hg : HD * (hg + 1)],
                                Wv[:, HD * hg : HD * (hg + 1)],
                            ],
                            axis=1,
                        ).astype(bf)
                    ),
                    "wo": np.ascontiguousarray(
                        Wo[DQ * hg : DQ * (hg + 1), :].astype(bf)
                    ),
                    "cs": cs,
                    "sc": sc,
                    "masks": mask,
                    "ident": ident,
                }
            )
    return in_maps


def run_on_device(x, cos, sin, Wq, Wk, Wv, Wo, trace=False):
    from concourse.bass_utils import run_bass_kernel_spmd

    T, C, NHL, BL = 2048, 2048, 4, 2
    in_maps = prepare_in_maps(x, cos, sin, Wq, Wk, Wv, Wo, T, C, NHL, BL)
    nc = build_nc(T, C, NHL, BL)
    res = run_bass_kernel_spmd(nc, in_maps, list(range(8)), trace=trace)

    B = x.shape[0]
    out = np.zeros((B, T, C), np.float32)
    n_hgrp = len(in_maps) // (B // BL)
    for g in range(B // BL):
        acc = np.zeros((BL * T, C), np.float32)
        for hg in range(n_hgrp):
            acc += res.results[g * n_hgrp + hg]["out"].astype(np.float32)
        out[BL * g : BL * (g + 1)] = acc.reshape(BL, T, C)
    return out, res


def kernel(x, cos, sin, Wq, Wk, Wv, Wo):
    out, _ = run_on_device(
        np.asarray(x), np.asarray(cos), np.asarray(sin),
        np.asarray(Wq), np.asarray(Wk), np.asarray(Wv), np.asarray(Wo),
    )
    return out


# revision 36
# speedup vs baseline: 1.2643x; 1.0013x over previous
"""Causal self-attention (GQA, RoPE, QK-RMSNorm) Trainium2 Bass kernel.

Sharding: tensor-parallel over heads x data-parallel over batch.
8 cores = 2 batch-groups (2 batches each) x 4 head-groups
(4 q heads + 1 kv head per core, GQA nrep=4).

Per core:
  - phase A: QKV projection (x^T via DMA-transpose as stationary operand),
    RoPE + RMSNorm in natural [t, d] layout, PE-transpose q,k to [d, t].
  - phase B: per (batch, j-chunk, head) "PJ": s^T = k^T.T @ q^T tiles ->
    exp on ACT (no max subtraction; rmsnormed q,k bound scores) with
    causally-trimmed widths -> boundary-block mask multiply -> E_sum
    (bf16 DVE adds) -> denominators via Pool partition_all_reduce ->
    reciprocal. AV is v-STATIONARY: psum[d, tq512] accumulates
    v[t0].T @ et[t0] over t0 (y^T lands pre-transposed, no ones column,
    no per-tqc psum juggling); per-colblock stop-flag splits keep the
    diagonal accumulation trimmed. yT_sb write folds the 1/D multiply.
  - phase C: partial out = y^T.T @ Wo_shard -> DRAM.
Host sums the 4 head-group partials per batch-group.

All matmuls bf16 inputs / fp32 PSUM accumulation; softmax, RoPE and
RMSNorm arithmetic in fp32 (E_sum accumulates in bf16; its ~0.3% rms
error on D is well inside the 2e-2 gate). rsqrt computed as
exp(-0.5*ln(x)) so every ACT call stays in one table set.
"""

import sys
from contextlib import ExitStack

import numpy as np

sys.path.insert(0, "/opt/trn_rl_repo")

import concourse.bass as bass  # noqa: E402
import concourse.bass_isa as bass_isa  # noqa: E402
import concourse.mybir as mybir  # noqa: E402
import concourse.tile as tile  # noqa: E402
from concourse import bacc  # noqa: E402

FP32 = mybir.dt.float32
BF16 = mybir.dt.bfloat16
P = 128
HD = 128
HD2 = HD // 2
TQW = 512  # tq chunk width for the QK stage


def build_nc(T=2048, C=2048, NHL=4, BL=2, repeat=1, et_bufs=2,
             xt_bufs=2, outsb_bufs=2, ps_s_bufs=3, ps_mm_bufs=2,
             drain_per_tile=1, no_qkt=0, no_av=0, no_esum=0, no_dchain=0,
             no_mask=0, qkv_interleave=0, elide=1, adaptive_drain=1):
    """Build the per-core Bass program. Same program on all 8 cores."""
    TL = BL * T
    NCC = C // P  # contraction chunks
    NTB = T // P  # t-128 chunks per batch
    DQ = NHL * HD
    NJ = T // TQW  # tq-512 chunks per batch
    NK = T // P  # tk-128 chunks per batch
    NCO = C // 512
    sm_scale = float(1.0 / np.sqrt(HD))
    EPS = 1e-6

    nc = bacc.Bacc(None)
    x = nc.declare_dram_parameter("x", [C, TL], BF16, isOutput=False)
    wq = nc.declare_dram_parameter("wq", [C, DQ], BF16, isOutput=False)
    wkv = nc.declare_dram_parameter("wkv", [C, 2 * HD], BF16, isOutput=False)
    wo = nc.declare_dram_parameter("wo", [DQ, C], BF16, isOutput=False)
    cs = nc.declare_dram_parameter("cs", [T, HD], FP32, isOutput=False)
    sc = nc.declare_dram_parameter("sc", [T, HD], FP32, isOutput=False)
    masks = nc.declare_dram_parameter("masks", [P, P], BF16, isOutput=False)
    ident = nc.declare_dram_parameter("ident", [P, P], BF16, isOutput=False)
    out = nc.declare_dram_parameter("out", [TL, C], BF16, isOutput=True)

    Exp = mybir.ActivationFunctionType.Exp
    Ln = mybir.ActivationFunctionType.Ln
    xr = x.rearrange("(n p) t -> p n t", p=P)  # x is shipped pre-transposed [C, TL]

    with tile.TileContext(nc) as tc, ExitStack() as ctx:
        const = ctx.enter_context(tc.tile_pool(name="const", bufs=1))
        persist = ctx.enter_context(tc.tile_pool(name="persist", bufs=1))
        xt_pool = ctx.enter_context(tc.tile_pool(name="xt", bufs=xt_bufs))
        work = ctx.enter_context(tc.tile_pool(name="work", bufs=2))
        et_pool = ctx.enter_context(tc.tile_pool(name="et", bufs=et_bufs))
        outsb = ctx.enter_context(tc.tile_pool(name="outsb", bufs=outsb_bufs))
        ps_mm = ctx.enter_context(tc.tile_pool(name="psmm", bufs=ps_mm_bufs, space="PSUM"))
        ps_s = ctx.enter_context(tc.tile_pool(name="pss", bufs=ps_s_bufs, space="PSUM"))
        ps_y = ctx.enter_context(tc.tile_pool(name="psy", bufs=2, space="PSUM"))

        # ---- constants / persistent state ----
        wq_sb = const.tile([P, NCC, DQ], BF16)
        wqr = wq.rearrange("(n p) d -> p n d", p=P)
        nc.scalar.dma_start(wq_sb[:, 0 : NCC // 4, :], wqr[:, 0 : NCC // 4, :])
        nc.scalar.dma_start(wq_sb[:, NCC // 4 :, :], wqr[:, NCC // 4 :, :])
        wkv_sb = const.tile([P, NCC, 2 * HD], BF16)
        nc.scalar.dma_start(wkv_sb[:], wkv.rearrange("(n p) d -> p n d", p=P))
        cs_sb = const.tile([P, NTB, HD], FP32)
        nc.scalar.dma_start(cs_sb[:], cs.rearrange("(n p) d -> p n d", p=P))
        sc_sb = const.tile([P, NTB, HD], FP32)
        nc.scalar.dma_start(sc_sb[:], sc.rearrange("(n p) d -> p n d", p=P))
        ident_sb = const.tile([P, P], BF16)
        nc.scalar.dma_start(ident_sb[:], ident[:])
        mask_sb = const.tile([P, P], BF16)
        nc.scalar.dma_start(mask_sb[:], masks[:])
        ones_sb = const.tile([P, P], BF16)  # ones col [:,0:1] + row [0:1,:]
        nc.vector.memset(ones_sb[:], 1.0)
        wo_sb = const.tile([P, NHL, C], BF16)
        nc.scalar.dma_start(wo_sb[:], wo.rearrange("(h p) c -> p h c", p=P))
        eps_sb = const.tile([P, 1], FP32)
        nc.vector.memset(eps_sb[:], EPS)

        qT_sb = persist.tile([P, BL * NHL, T], BF16)  # [d, plane, t]
        kT_sb = persist.tile([P, BL, T], BF16)
        v_sb = persist.tile([P, BL, NK, HD], BF16)  # [tk_in, b, tk_chunk, d]
        yT_sb = persist.tile([P, BL * NHL, T], BF16)

        if no_qkt:
            nc.vector.memset(qT_sb[:], 0.0)
            nc.vector.memset(kT_sb[:], 0.0)
        if no_av:
            nc.vector.memset(yT_sb[:], 0.0)
        rcp_const = None
        if no_dchain:
            rcp_const = persist.tile([P, TQW], FP32)
            nc.vector.memset(rcp_const[:], 1.0)
        if no_esum:
            es_init = persist.tile([P, TQW], BF16)
            nc.vector.memset(es_init[:], 1.0)

        rep_ctx = tc.For_i(0, repeat, 1) if repeat > 1 else None
        if rep_ctx is not None:
            ctx.enter_context(rep_ctx)

        # ---- phase A iteration: projection + rope + rmsnorm + transpose ----
        def a_iter(b, t16):
            trow = b * T + t16 * P
            xt = xt_pool.tile([P, NCC, P], BF16, tag="xt")
            nc.sync.dma_start(xt[:], xr[:, :, trow : trow + P])

            psq = ps_mm.tile([P, 512], FP32, tag="mm512")
            pskv = ps_mm.tile([P, 512], FP32, tag="mm512")
            if qkv_interleave:
                for c in range(NCC):
                    nc.tensor.matmul(
                        psq[:, :DQ], xt[:, c, :], wq_sb[:, c, :],
                        start=(c == 0), stop=(c == NCC - 1),
                    )
                    nc.tensor.matmul(
                        pskv[:, 0 : 2 * HD], xt[:, c, :], wkv_sb[:, c, :],
                        start=(c == 0), stop=(c == NCC - 1),
                    )
            else:
                for c in range(NCC):
                    nc.tensor.matmul(
                        psq[:, :DQ], xt[:, c, :], wq_sb[:, c, :],
                        start=(c == 0), stop=(c == NCC - 1),
                    )
                for c in range(NCC):
                    nc.tensor.matmul(
                        pskv[:, 0 : 2 * HD], xt[:, c, :], wkv_sb[:, c, :],
                        start=(c == 0), stop=(c == NCC - 1),
                    )

            # rope: p1 = [x1*cos | x2*sin], p2 = [x1*sin | x2*cos]
            # y1 = p1_lo + p1_hi ; y2 = p2_hi - p2_lo
            psq3 = psq[:, 0:DQ].rearrange("p (h d) -> p h d", d=HD)
            csb = cs_sb[:, t16, None, :].to_broadcast((P, NHL, HD))
            scb = sc_sb[:, t16, None, :].to_broadcast((P, NHL, HD))
            p1 = work.tile([P, NHL, HD], FP32, tag="p1")
            p2 = work.tile([P, NHL, HD], FP32, tag="p2")
            nc.vector.tensor_mul(p1[:], psq3, csb)
            nc.vector.tensor_mul(p2[:], psq3, scb)
            q_ro = work.tile([P, NHL, HD], FP32, tag="qro")
            nc.vector.tensor_add(
                q_ro[:, :, 0:HD2], p1[:, :, 0:HD2], p1[:, :, HD2:HD]
            )
            nc.vector.tensor_sub(
                q_ro[:, :, HD2:HD], p2[:, :, HD2:HD], p2[:, :, 0:HD2]
            )
            p1k = work.tile([P, HD], FP32, tag="p1k")
            p2k = work.tile([P, HD], FP32, tag="p2k")
            nc.vector.tensor_mul(p1k[:], pskv[:, 0:HD], cs_sb[:, t16, :])
            nc.vector.tensor_mul(p2k[:], pskv[:, 0:HD], sc_sb[:, t16, :])
            k_ro = work.tile([P, HD], FP32, tag="kro")
            nc.vector.tensor_add(k_ro[:, 0:HD2], p1k[:, 0:HD2], p1k[:, HD2:HD])
            nc.vector.tensor_sub(k_ro[:, HD2:HD], p2k[:, HD2:HD], p2k[:, 0:HD2])

            # rmsnorm scales: rs = exp(-0.5 * ln(ssq/HD + eps))
            sq = work.tile([P, NHL, HD], FP32, tag="p1")  # reuse p1 ring (SBUF)
            nc.vector.tensor_mul(sq[:], q_ro[:], q_ro[:])
            sqk = work.tile([P, HD], FP32, tag="sqk")
            nc.vector.tensor_mul(sqk[:], k_ro[:], k_ro[:])
            ssq = work.tile([P, NHL + 1], FP32, tag="ssq")
            nc.vector.tensor_reduce(
                ssq[:, 0:NHL], sq[:], axis=mybir.AxisListType.X,
                op=mybir.AluOpType.add,
            )
            nc.vector.tensor_reduce(
                ssq[:, NHL : NHL + 1], sqk[:], axis=mybir.AxisListType.X,
                op=mybir.AluOpType.add,
            )
            lnv = work.tile([P, NHL + 1], FP32, tag="lnv")
            nc.scalar.activation(
                lnv[:], ssq[:], Ln, bias=eps_sb[:, 0:1], scale=1.0 / HD
            )
            rs = work.tile([P, NHL + 1], FP32, tag="rs")
            nc.scalar.activation(rs[:], lnv[:], Exp, scale=-0.5)

            q_n = work.tile([P, NHL, HD], BF16, tag="qn")
            nc.vector.tensor_mul(
                q_n[:], q_ro[:], rs[:, 0:NHL, None].to_broadcast((P, NHL, HD))
            )
            k_n = work.tile([P, HD], BF16, tag="kn")
            nc.vector.tensor_scalar_mul(k_n[:], k_ro[:], rs[:, NHL : NHL + 1])

            nc.vector.tensor_copy(v_sb[:, b, t16, 0:HD], pskv[:, HD : 2 * HD])

            if no_qkt:
                return
            pt = ps_y.tile([P, NHL + 1, 132], BF16, tag="ptq", bufs=1)
            for h in range(NHL):
                nc.tensor.transpose(pt[:, h, 0:P], q_n[:, h, :], ident_sb[:])
            nc.tensor.transpose(pt[:, NHL, 0:P], k_n[:], ident_sb[:])
            nc.any.tensor_copy(
                qT_sb[:, b * NHL : b * NHL + NHL, t16 * P : (t16 + 1) * P],
                pt[:, 0:NHL, 0:P],
            )
            nc.any.tensor_copy(
                kT_sb[:, b, t16 * P : (t16 + 1) * P], pt[:, NHL, 0:P]
            )

        # ---- phase C: one output-column chunk of Wo for one (b, t16) ----
        def c_co(b, t16, co):
            pso = ps_mm.tile([P, 512], FP32, tag="mm512", name=f"pso{co}")
            for h in range(NHL):
                nc.tensor.matmul(
                    pso[:],
                    yT_sb[:, b * NHL + h, t16 * P : (t16 + 1) * P],
                    wo_sb[:, h, co * 512 : (co + 1) * 512],
                    start=(h == 0), stop=(h == NHL - 1),
                )
            o_sb = outsb.tile([P, 512], BF16, tag="osb")
            nc.scalar.activation(
                o_sb[:], pso[:], mybir.ActivationFunctionType.Copy
            )
            trow = b * T + t16 * P
            nc.sync.dma_start(
                out[trow : trow + P, co * 512 : (co + 1) * 512], o_sb[:]
            )

        # ---- phase B: scores+exp+denominator for one PJ=(b,j,h) ----
        def b_scores(b, j, h, av_gen):
            """Emit score matmuls + exp + mask + E_sum for PJ (b,j,h),
            interleaving the pending AV matmul stream (av_gen) of the
            previous PJ so PE has work while ACT chews through the exps.
            Returns (et_slot, es_tile, av_gen)."""
            plane = b * NHL + h
            ntiles = 4 * j + 4
            et = et_pool.tile([P, 4 * NJ, TQW], BF16, tag="et")
            es = es_init if no_esum else work.tile([P, TQW], BF16, tag="esum")
            for t0 in range(ntiles):
                m = t0 - 4 * j
                c0 = max(m, 0) * P
                ps = ps_s.tile([P, TQW], FP32, tag="s")
                nc.tensor.matmul(
                    ps[:, c0:TQW],
                    kT_sb[:, b, t0 * P : (t0 + 1) * P],
                    qT_sb[:, plane, j * TQW + c0 : (j + 1) * TQW],
                    start=True, stop=True,
                )
                nc.scalar.activation(
                    et[:, t0, c0:TQW], ps[:, c0:TQW], Exp, scale=sm_scale
                )
                if m >= 0 and not no_mask:
                    nc.vector.tensor_mul(
                        et[:, t0, c0 : c0 + P], et[:, t0, c0 : c0 + P],
                        mask_sb[:],
                    )
                if not no_esum:
                    if t0 == 0:
                        nc.vector.tensor_copy(es[:], et[:, 0, :])
                    else:
                        nc.vector.tensor_add(
                            es[:, c0:TQW], es[:, c0:TQW], et[:, t0, c0:TQW]
                        )
                if av_gen is not None:
                    n_drain = drain_per_tile + (
                        1 if adaptive_drain and t0 >= ntiles - 5 else 0
                    )
                    for _ in range(n_drain):
                        if next(av_gen, None) is None:
                            av_gen = None
                            break
            return et, es, av_gen

        def b_av_gen(b, j, h, et, es):
            """v-stationary AV: psum[d, tq512] accumulates v[t0].T @ et[t0]
            with causal trimming; diagonal tiles split per colblock so the
            stop flags close each column range exactly once. The denominator
            chain is emitted at the END of this stream (one PJ later in the
            PE queue) so the in-order PE never stalls on the exp/mask/es
            tail: ONE all-ones [128,128] stationary matmul both collapses
            the partition dim and broadcasts (out[m,n] = D[n] for every m),
            then reciprocal, then the 1/D-folded copy into yT_sb. (A gpsimd
            allreduce is ~5us/call on HW; a PE->ACT->PE chain stalls the
            in-order ACT queue.)"""
            plane = b * NHL + h
            ntiles = 4 * j + 4
            psy = ps_y.tile([P, TQW], FP32, tag="yt")
            if no_av:
                return
            rcp = rcp_const
            for t0 in range(ntiles):
                m = t0 - 4 * j
                c0 = max(m, 0) * P
                first = t0 == 0
                vst = v_sb[:, b, t0, :]
                if 0 <= m < 3:
                    nc.tensor.matmul(
                        psy[:, c0 : c0 + P], vst, et[:, t0, c0 : c0 + P],
                        start=first, stop=True,
                    )
                    yield True
                    nc.tensor.matmul(
                        psy[:, c0 + P : TQW], vst, et[:, t0, c0 + P : TQW],
                        start=first, stop=False,
                    )
                    yield True
                elif m == 3:
                    nc.tensor.matmul(
                        psy[:, c0:TQW], vst, et[:, t0, c0:TQW],
                        start=first, stop=True,
                    )
                    yield True
                else:
                    nc.tensor.matmul(
                        psy[:], vst, et[:, t0, :], start=first, stop=False,
                    )
                    yield True
            if not no_dchain:
                bc_ps = ps_s.tile([P, TQW], FP32, tag="s", name="bcps")
                nc.tensor.matmul(
                    bc_ps[:], ones_sb[:], es[:], start=True, stop=True
                )
                # 1/D as exp(-ln(D)) on ACT: DVE reciprocal of [128,512] is
                # multi-pass on HW (~1.2us/PJ); ACT has slack and its Ln
                # read frees the borrowed ps_s slot quickly.
                lnd = work.tile([P, TQW], FP32, tag="lnd")
                nc.scalar.activation(lnd[:], bc_ps[:], Ln)
                rcp = work.tile([P, TQW], FP32, tag="rcp")
                nc.scalar.activation(rcp[:], lnd[:], Exp, scale=-1.0)
                yield True
            nc.vector.tensor_mul(
                yT_sb[:, plane, j * TQW : (j + 1) * TQW], psy[:], rcp[:]
            )
            yield True

        # ---- pipelined schedule ----
        # prologue: first A window of batch 0
        for t16 in range(4):
            a_iter(0, t16)

        c_queue = []  # ready (b, t16, co) phase-C chunks
        c_staged = []  # chunks whose yT just landed; promoted next iter
        pending = None  # (av_gen, b, j) of the previous PJ
        plane_order = [(b, j, h) for b in range(BL) for j in range(NJ)
                       for h in range(NHL)]

        def drain_pending(push_c):
            nonlocal pending
            if pending is None:
                return
            gen, pb, pj = pending
            if gen is not None:
                for _ in gen:
                    pass
            pending = None
            if push_c:
                for t16 in range(4 * pj, 4 * pj + 4):
                    for co in range(NCO):
                        c_staged.append((pb, t16, co))

        for b, j, h in plane_order:
            c_queue.extend(c_staged)
            del c_staged[:]
            av = pending[0] if pending is not None else None
            et, es, av = b_scores(b, j, h, av)
            if pending is not None:
                pending = (av, pending[1], pending[2])
                drain_pending(push_c=(h == 0))  # prev PJ was (b', j', h=3)
            pending = (b_av_gen(b, j, h, et, es), b, j)

            # filler slot: phase A iteration and/or phase C chunks
            if b == 0 and j < 3:
                a_iter(0, 4 * (j + 1) + h)
            elif b == 0 and j == 3:
                a_iter(1, h)
            elif b == 1 and j < 3:
                a_iter(1, 4 * (j + 1) + h)
            n_c = 3 if b == 0 else (4 if j < 3 else 6)
            for _ in range(min(n_c, len(c_queue))):
                c_co(*c_queue.pop(0))

        drain_pending(push_c=True)
        c_queue.extend(c_staged)
        del c_staged[:]
        while c_queue:
            c_co(*c_queue.pop(0))

    nc.finalize()
    _collapse_act_table_loads(nc)
    if elide:
        _elide_redundant_ldweights(nc)
    return nc


def _ldw_sig(inst):
    if not inst.ins:
        return None
    ap = inst.ins[0]
    return (
        getattr(ap, "memref", None),
        getattr(ap, "offset", None),
        str(getattr(ap, "ap", None)),
        str(getattr(ap, "dtype", None)),
        str(inst.perf_mode),
        str(inst.is_transpose),
        str(inst.tile_position),
    )


def _elide_redundant_ldweights(nc):
    """Drop an InstLdweights whose weights signature matches the PE array's
    current contents (loaded by the previous Ldweights, untouched by the
    intervening Matmults). Only sync-free loads are dropped: any write to
    the weights region between the two loads would impose a RAW wait on
    the candidate, which keeps it."""
    for fn in nc.m.functions:
        for bb in fn.blocks:
            kept = []
            changed = False
            last_sig = None
            for inst in bb.instructions:
                nm = type(inst).__name__
                if nm == "InstLdweights":
                    sig = _ldw_sig(inst)
                    si = inst.sync_info
                    has_sync = si is not None and (
                        bool(si.on_wait) or bool(si.on_update)
                    )
                    if sig is not None and sig == last_sig and not has_sync:
                        changed = True
                        continue
                    last_sig = sig
                kept.append(inst)
            if changed:
                del bb.instructions[:]
                for i in kept:
                    bb.instructions.append(i)


def _collapse_act_table_loads(nc):
    """Every ACT function used here (Exp, Ln, Copy, Identity, Square) lives in
    the natural_log_exp_and_others set, but the insertion pass alternates
    exp_and_others / natural_log (one ~1.3us reload per rsqrt) — rewrite the
    first load to the combined set and drop the redundant reloads."""
    from concourse.hw_specs import get_activation_tables

    tables = list(get_activation_tables(nc.m.arch))
    target = tables.index("natural_log_exp_and_others")
    first = True
    for fn in nc.m.functions:
        for bb in fn.blocks:
            kept = []
            changed = False
            for inst in bb.instructions:
                if type(inst).__name__ == "InstLoadActFuncSet":
                    assert inst.sync_info is None
                    if first:
                        inst.act_func_set_id = target
                        first = False
                        kept.append(inst)
                    else:
                        changed = True
                    continue
                kept.append(inst)
            if changed:
                del bb.instructions[:]
                for i in kept:
                    bb.instructions.append(i)


def make_host_consts(T, dtype_bf):
    """Boundary causal mask (keep cc >= i) + PE-transpose identity."""
    i = np.arange(P)[:, None]
    cc = np.arange(P)[None, :]
    mask = (i <= cc).astype(np.float32).astype(dtype_bf)
    ident = np.eye(P, dtype=np.float32).astype(dtype_bf)
    return mask, ident


def prepare_in_maps(x, cos, sin, Wq, Wk, Wv, Wo, T=2048, C=2048, NHL=4, BL=2):
    import ml_dtypes

    bf = ml_dtypes.bfloat16
    B = x.shape[0]
    n_bgrp = B // BL
    n_hgrp = (Wq.shape[1] // HD) // NHL
    DQ = NHL * HD

    x_bf = np.ascontiguousarray(x.astype(bf))
    cosf = np.ascontiguousarray(cos.reshape(T, HD2).astype(np.float32))
    sinf = np.ascontiguousarray(sin.reshape(T, HD2).astype(np.float32))
    cs = np.ascontiguousarray(np.concatenate([cosf, sinf], axis=1))
    sc = np.ascontiguousarray(np.concatenate([sinf, cosf], axis=1))
    mask, ident = make_host_consts(T, bf)

    in_maps = []
    for g in range(n_bgrp):
        x_sh = np.ascontiguousarray(x_bf[BL * g : BL * (g + 1)].reshape(BL * T, C).T)
        for hg in range(n_hgrp):
            in_maps.append(
                {
                    "x": x_sh,
                    "wq": np.ascontiguousarray(
                        Wq[:, DQ * hg : DQ * (hg + 1)].astype(bf)
                    ),
                    "wkv": np.ascontiguousarray(
                        np.concatenate(
                            [
                                Wk[:, HD * hg : HD * (hg + 1)],
                                Wv[:, HD * hg : HD * (hg + 1)],
                            ],
                            axis=1,
                        ).astype(bf)
                    ),
                    "wo": np.ascontiguousarray(
                        Wo[DQ * hg : DQ * (hg + 1), :].astype(bf)
                    ),
                    "cs": cs,
                    "sc": sc,
                    "masks": mask,
                    "ident": ident,
                }
            )
    return in_maps


def run_on_device(x, cos, sin, Wq, Wk, Wv, Wo, trace=False):
    from concourse.bass_utils import run_bass_kernel_spmd

    T, C, NHL, BL = 2048, 2048, 4, 2
    in_maps = prepare_in_maps(x, cos, sin, Wq, Wk, Wv, Wo, T, C, NHL, BL)
    nc = build_nc(T, C, NHL, BL)
    res = run_bass_kernel_spmd(nc, in_maps, list(range(8)), trace=trace)

    B = x.shape[0]
    out = np.zeros((B, T, C), np.float32)
    n_hgrp = len(in_maps) // (B // BL)
    for g in range(B // BL):
        acc = np.zeros((BL * T, C), np.float32)
        for hg in range(n_hgrp):
            acc += res.results[g * n_hgrp + hg]["out"].astype(np.float32)
        out[BL * g : BL * (g + 1)] = acc.reshape(BL, T, C)
    return out, res


def kernel(x, cos, sin, Wq, Wk, Wv, Wo):
    out, _ = run_on_device(
        np.asarray(x), np.asarray(cos), np.asarray(sin),
        np.asarray(Wq), np.asarray(Wk), np.asarray(Wv), np.asarray(Wo),
    )
    return out
